# revision 53
# baseline (speedup 1.0000x reference)
"""Trainium2 Bass kernel for the BTST-SSM problem.

Math: 2D state-space model. Per l: u -> conv(B) -> DST-eigendomain ->
diagonal linear recurrence over l -> inverse DST -> conv(C) + conv(D) -> gelu.

Sharding: batch (8) across 8 cores, one sample per core. No collectives.

The warm call is axon-tunnel-bandwidth bound (~30-40 MB/s, half duplex), so
the design minimizes wire bytes and per-RPC overheads:
  - useq/x0 ship as int8 (per-core scale folded into that core's B/D conv
    weight copies; dequant on device is a free int8->fp16 copy)
  - conv weights ship as fp16; y returns as int8 with per-l scales computed
    on device (vector reduce_max + gpsimd partition_all_reduce)
  - all per-core inputs ride in ONE int8 wire tensor (fp16/f32 segments via
    AP.bitcast), packed per core in threads that overlap the serial uploads
  - A_bar/B_coeff are computed on device from tiny per-channel vectors:
    A^(1/32) via Exp/Sin (args stay in the accurate range) then 5 complex
    squarings; B = (A-1)*conj(temp)/|temp|^2 with vector.reciprocal
  - input-independent DST/basis constants are baked into the NEFF via
    inline_tensor (shipped once at executable load)
  - custom cached-jit runner built directly on _bass_exec_p: no per-call
    retrace, no zero donation buffers, fast-dispatch compile
  - y shards are fetched + dequantized concurrently (8 threads)

Per-core layouts (SBUF tiles are (128 partitions, free)):
  - channel-major image: (ch, h*w) padded to (ch, 34*34) for SAME conv
  - spatial tiles for transforms: partition = (g4, x32) block-diag groups
  - scan state: partition q = dc*32 + a  (c = 4u + dc), free col = u*64 + p
All transform matrices are packed host-side as kron(I4, blk) lhsT tensors;
complex arithmetic is done with +/- weight copies accumulated in PSUM.

Accuracy: ~1.43e-2 max-relative (dominated by int8 input quantization,
validated against the reference in the numpy mock; deterministic).
"""

import os
import sys
import numpy as np

sys.path.insert(0, "/opt/trn_rl_repo")

H = W = 32
P = 64
U = 64
L = 32
BSZ = 8
PW = 34          # padded width/height
NPIX = H * W     # 1024
NPAD = PW * PW   # 1156
PI = float(np.pi)


# ----------------------------------------------------------------------------
# Host constant computation (float64 -> float32)
# ----------------------------------------------------------------------------

def _dst_q(n):
    idx = np.arange(1, n + 1, dtype=np.float64)
    s = np.sin(np.pi * idx[:, None] * idx[None, :] / (n + 1)) / np.sqrt((n + 1) / 2.0)
    phase = np.exp(1j * (np.pi / 2.0) * idx)
    return phase[:, None] * s


def _softmax(x, axis):
    m = np.max(x, axis=axis, keepdims=True)
    e = np.exp(x - m)
    return e / np.sum(e, axis=axis, keepdims=True)


def static_constants():
    """Input-independent constants baked into the NEFF as inline tensors."""
    Qh = _dst_q(H)
    Qw = _dst_q(W)
    Qh_inv = np.conj(Qh).T
    Qw_inv = np.conj(Qw).T

    def kr(m):
        return np.kron(np.eye(4), m).astype(np.float32)

    ih = np.arange(1, H + 1, dtype=np.float64)
    iw = np.arange(1, W + 1, dtype=np.float64)
    ch = 2.0 * np.cos(np.pi * ih / (H + 1))
    cw = 2.0 * np.cos(np.pi * iw / (W + 1))
    # packed (q=(dc,a), col=(u,p)) grids of cw[c]=cw[4u+dc] and ch[a]
    cw_pk = np.zeros((128, 512), dtype=np.float64)
    ch_pk = np.zeros((128, 512), dtype=np.float64)
    for dc in range(4):
        for uu in range(8):
            cw_pk[dc * 32:(dc + 1) * 32, uu * 64:(uu + 1) * 64] = cw[4 * uu + dc]
    for a in range(32):
        ch_pk[a::32, :] = ch[a]

    return {
        "qwf_r": kr(np.real(Qw_inv).T), "qwf_i": kr(np.imag(Qw_inv).T),
        "qwf_ni": kr(-np.imag(Qw_inv).T),
        "qhf_r": kr(np.real(Qh_inv).T), "qhf_i": kr(np.imag(Qh_inv).T),
        "qhf_ni": kr(-np.imag(Qh_inv).T),
        "qhi_r": kr(np.real(Qh).T), "qhi_i": kr(np.imag(Qh).T),
        "qhi_ni": kr(-np.imag(Qh).T),
        "qwi_r": kr(np.real(Qw).T), "qwi_i": kr(np.imag(Qw).T),
        "qwi_ni": kr(-np.imag(Qw).T),
        "ident": np.eye(128, dtype=np.float32),
        "ident16": np.eye(128, dtype=np.float16),
        "ones1": np.ones((1, 128), dtype=np.float32),
        "cw_pk": cw_pk.astype(np.float32),
        "ch_pk": ch_pk.astype(np.float32),
    }


PAIRS = [((0, 0), (0, 1)), ((1, 0), (1, 1)), ((2, 0), (2, 1)), ((0, 2), (1, 2))]
PAIR_OFFS = [0, 34, 68, 2]      # window offset of t0 per pair group
PAIR_DELTA = [1, 1, 1, 34]      # o(t1) - o(t0); delta 1 -> up2d1, 34 -> up2d34
SOLO_OFF = 70                   # (2,2)

# single int8 wire blob per core: quantized useq+x0 (per-core scale folded
# into that core's B/D conv weights), fp16 conv weights, and f32 pvec — one
# large axon transfer instead of many. Offsets are in BYTES, 64-aligned.
_SEGS8 = [("useq", L * NPIX * U), ("x0", NPIX * U)]          # int8, 1B each
_SEGS16 = [("wbp", 128 * 4 * 128), ("wdp", 128 * 4 * 64), ("wbs", 64 * 128),
           ("wds", 64 * 64), ("wc", 128 * 9 * 64)]           # fp16, 2B each
BLOB_OFF = {}
_o = 0
for _n, _s in _SEGS8:
    BLOB_OFF[_n] = _o
    _o += _s
for _n, _s in _SEGS16:
    BLOB_OFF[_n] = _o
    _o += 2 * _s
BLOB_OFF["pvec"] = _o
_o += 6 * 512 * 4
NWIRE = _o


def pack_dynamic(Lambda_re, Lambda_im, values, log_step, B_r, B_i, C_r, C_i,
                 D_kernel):
    """Per-call small tensors: channel vectors + fp16-packed conv weights."""
    lam_re = np.minimum(Lambda_re.astype(np.float64), -1e-4)
    lam_im = Lambda_im.astype(np.float64)
    step = np.exp(log_step.astype(np.float64))
    v = _softmax(values.astype(np.float64), axis=-1) * 4.0
    xk, yk, zk, wk = v[:, 0], v[:, 1], v[:, 2], v[:, 3]
    kv = np.stack(((xk + yk - 2) / 4, (xk + zk - 2) / 4, (xk + wk - 2) / 8),
                  axis=-1)                                     # (P, 3)
    pvec = np.zeros((6, 512), dtype=np.float64)
    rows = [lam_re, lam_im, step, kv[:, 0], kv[:, 1], kv[:, 2]]
    for r, vec in enumerate(rows):
        pvec[r] = np.tile(vec, 8)
    pvec = pvec.astype(np.float32)

    wb = np.concatenate([B_r, B_i], axis=-1).transpose(2, 0, 1, 3) \
        .reshape(U, 9, 2 * P)
    wc = np.concatenate([2.0 * C_r, -2.0 * C_i], axis=2).transpose(2, 0, 1, 3) \
        .reshape(2 * P, 9, U).astype(np.float16)
    wd = D_kernel.transpose(2, 0, 1, 3).reshape(U, 9, U)
    wb9 = wb.reshape(U, 3, 3, 2 * P)
    wd9 = wd.reshape(U, 3, 3, U)

    def pack_pairs(w9, cout):
        out = np.zeros((128, 4, cout), dtype=np.float16)
        for g, (t0, t1) in enumerate(PAIRS):
            out[0:64, g, :] = w9[:, t0[0], t0[1], :]
            out[64:128, g, :] = w9[:, t1[0], t1[1], :]
        return out

    return {
        "pvec": pvec,
        "wbp": pack_pairs(wb9, 2 * P),
        "wdp": pack_pairs(wd9, U),
        "wbs": np.ascontiguousarray(wb9[:, 2, 2, :]).astype(np.float16),
        "wds": np.ascontiguousarray(wd9[:, 2, 2, :]).astype(np.float16),
        "wc": wc,
    }


# ----------------------------------------------------------------------------
# Host reference of the device A_bar/B_coeff computation (for validation)
# ----------------------------------------------------------------------------

def abc_host(pvec, cst):
    """Mirror of the device prologue, in numpy f32."""
    lam_re = pvec[0][None, :]      # broadcast rows (128, 512)
    lam_im = pvec[1][None, :]
    step = pvec[2][None, :]
    kv0, kv1, kv2 = pvec[3][None, :], pvec[4][None, :], pvec[5][None, :]
    CW, CH = cst["cw_pk"], cst["ch_pk"]
    D = (kv2 * CW + kv1) * CH + kv0 * CW + 1.0
    tr = lam_re * D
    ti = lam_im * D
    mr = tr * step
    mi = ti * step
    er32 = np.exp(mr / 32.0)
    s32 = np.sin(mi / 32.0)
    s64 = np.sin(mi / 64.0)
    c32 = 1.0 - 2.0 * s64 * s64
    ar, ai = er32 * c32, er32 * s32
    for _ in range(5):
        ar, ai = ar * ar - ai * ai, 2.0 * ar * ai
    d2 = tr * tr + ti * ti
    inv = 1.0 / d2
    arm1 = ar - 1.0
    bc_r = (arm1 * tr + ai * ti) * inv
    bc_i = (ai * tr - arm1 * ti) * inv
    return ar, ai, bc_r, bc_i


# ----------------------------------------------------------------------------
# Bass kernel
# ----------------------------------------------------------------------------

def build_bass():
    import concourse.bass as bass
    import concourse.bacc as bacc
    import concourse.mybir as mybir
    import concourse.tile as tile
    import concourse.bass_isa as bass_isa

    f32 = mybir.dt.float32
    f16 = mybir.dt.float16
    AF = mybir.ActivationFunctionType
    nc = bacc.Bacc(None)

    i8 = mybir.dt.int8
    # wire blob splits into two input tensors per core so each core exposes
    # two H2D buffers (more in-flight upload RPCs on the axon pipe)
    NSPLIT = (L // 2) * NPIX * U        # first half of useq
    bloba_d = nc.dram_tensor("bloba", [NSPLIT], i8, kind="ExternalInput")
    blobb_d = nc.dram_tensor("blobb", [NWIRE - NSPLIT], i8,
                             kind="ExternalInput")

    def wire(off, size):
        if off + size <= NSPLIT:
            return bloba_d[off:off + size]
        assert off >= NSPLIT
        return blobb_d[off - NSPLIT:off - NSPLIT + size]

    def seg8(name, size):
        return wire(BLOB_OFF[name], size)

    def seg16(name, size):
        return wire(BLOB_OFF[name], 2 * size).bitcast(f16)

    def seg32(name, size):
        return wire(BLOB_OFF[name], 4 * size).bitcast(f32)

    dyn_shapes = {
        "wbp": (128, 4, 128), "wdp": (128, 4, 64),
        "wbs": (64, 128), "wds": (64, 64),
        "wc": (128, 9, 64),
    }
    # y splits into two output tensors so each core exposes two D2H buffers
    # (more in-flight fetch RPCs on the axon pipe)
    ya_d = nc.dram_tensor("ya", [L // 2, NPIX, U], i8, kind="ExternalOutput")
    yb_d = nc.dram_tensor("yb", [L // 2, NPIX, U], i8, kind="ExternalOutput")
    ysc_d = nc.dram_tensor("ysc", [1, L], f32, kind="ExternalOutput")

    scst = static_constants()
    cst_dram = {k: nc.inline_tensor(v, name=k) for k, v in scst.items()}

    taps = [(kh, kw) for kh in range(3) for kw in range(3)]

    with tile.TileContext(nc) as tc:
        with (
            tc.tile_pool(name="cpool", bufs=1) as cpool,
            tc.tile_pool(name="state", bufs=1) as spool,
            tc.tile_pool(name="work", bufs=2) as work,
            tc.tile_pool(name="tmp", bufs=2) as tmp_pool,
            tc.tile_pool(name="pacc", bufs=1, space="PSUM") as pacc,
            tc.tile_pool(name="pt", bufs=2, space="PSUM") as pt_pool,
            tc.tile_pool(name="pw", bufs=2, space="PSUM") as pw_pool,
            tc.tile_pool(name="pbh", bufs=2, space="PSUM") as pbh_pool,
        ):
            cst = {}
            for k, v in scst.items():
                t = cpool.tile(list(v.shape), f16 if v.dtype == np.float16 else f32,
                               tag=k, name=k)
                nc.sync.dma_start(t[:], cst_dram[k][:])
                cst[k] = t
            for k, shp in dyn_shapes.items():
                t = cpool.tile(list(shp), f16, tag=k, name=k)
                nelem = int(np.prod(shp))
                src = seg16(k, nelem)
                if len(shp) == 2:
                    src = src.rearrange("(p a) -> p a", p=shp[0])
                else:
                    src = src.rearrange("(p a b) -> p a b", p=shp[0], a=shp[1])
                nc.sync.dma_start(t[:], src)
                cst[k] = t
            pvec_all = seg32("pvec", 6 * 512)
            pvec_rows = []
            for r in range(6):
                rt = cpool.tile([1, 512], f32, tag=f"pvec{r}", name=f"pvec_t{r}")
                nc.sync.dma_start(
                    rt[:], pvec_all[r * 512:(r + 1) * 512]
                    .rearrange("(o a) -> o a", o=1))
                pvec_rows.append(rt)

            # persistent scan state + zeroed padded buffers + A/Bc tiles
            s_r = spool.tile([128, 512], f32, tag="sr")
            s_i = spool.tile([128, 512], f32, tag="si")
            a_r = spool.tile([128, 512], f32, tag="a_r")
            a_i = spool.tile([128, 512], f32, tag="a_i")
            bc_r = spool.tile([128, 512], f32, tag="bc_r")
            bc_i = spool.tile([128, 512], f32, tag="bc_i")
            upad = spool.tile([64, NPAD], f16, tag="upad")
            cpad = spool.tile([128, NPAD], f16, tag="cpad")
            ds_all = spool.tile([1, L], f32, tag="ds_all")
            rmax = spool.tile([128, 1], f32, tag="rmax")
            nc.vector.memset(upad[:], 0.0)
            nc.vector.memset(cpad[:], 0.0)
            nc.vector.memset(rmax[:], 0.0)

            # ---------------- device prologue: A_bar / B_coeff ----------------
            def bcast(row):
                pb = pw_pool.tile([128, 512], f32, tag="pw")
                nc.tensor.matmul(pb[:], cst["ones1"][:],
                                 pvec_rows[row][:], start=True, stop=True)
                dst = spool.tile([128, 512], f32, tag=f"bc_row{row}")
                nc.scalar.copy(dst[:], pb[:])
                return dst

            lam_re_b = bcast(0)
            lam_im_b = bcast(1)
            step_b = bcast(2)
            kv0_b = bcast(3)
            kv1_b = bcast(4)
            kv2_b = bcast(5)

            t_d = spool.tile([128, 512], f32, tag="t_d")
            t_e = spool.tile([128, 512], f32, tag="t_e")
            tr = spool.tile([128, 512], f32, tag="t_tr")
            ti = spool.tile([128, 512], f32, tag="t_ti")
            # D = (kv2*CW + kv1)*CH + kv0*CW + 1
            nc.vector.tensor_mul(t_d[:], kv2_b[:], cst["cw_pk"][:])
            nc.vector.tensor_add(t_d[:], t_d[:], kv1_b[:])
            nc.vector.tensor_mul(t_d[:], t_d[:], cst["ch_pk"][:])
            nc.vector.tensor_mul(t_e[:], kv0_b[:], cst["cw_pk"][:])
            nc.vector.tensor_add(t_d[:], t_d[:], t_e[:])
            nc.vector.tensor_scalar_add(t_d[:], t_d[:], 1.0)
            # temp = lam * D (complex); m = temp*step
            nc.vector.tensor_mul(tr[:], lam_re_b[:], t_d[:])
            nc.vector.tensor_mul(ti[:], lam_im_b[:], t_d[:])
            mr = lam_re_b   # reuse row tiles as scratch
            mi = lam_im_b
            nc.vector.tensor_mul(mr[:], tr[:], step_b[:])
            nc.vector.tensor_mul(mi[:], ti[:], step_b[:])
            # A^(1/32) = exp(mr/32) * (1-2*sin(mi/64)^2, sin(mi/32))
            er32 = step_b   # scratch
            s32 = kv0_b
            s64 = kv1_b
            nc.scalar.activation(er32[:], mr[:], AF.Exp, scale=1.0 / 32.0)
            nc.scalar.activation(s32[:], mi[:], AF.Sin, scale=1.0 / 32.0)
            nc.scalar.activation(s64[:], mi[:], AF.Sin, scale=1.0 / 64.0)
            nc.vector.tensor_mul(t_e[:], s64[:], s64[:])
            nc.vector.tensor_scalar(t_e[:], t_e[:], -2.0, 1.0,
                                    op0=mybir.AluOpType.mult,
                                    op1=mybir.AluOpType.add)   # c32
            nc.vector.tensor_mul(a_r[:], er32[:], t_e[:])
            nc.vector.tensor_mul(a_i[:], er32[:], s32[:])
            # 5x complex squaring -> A_bar
            sq_r = kv2_b    # scratch
            sq_i = t_e
            for _ in range(5):
                nc.vector.tensor_mul(sq_r[:], a_r[:], a_r[:])
                nc.vector.tensor_mul(sq_i[:], a_i[:], a_i[:])
                nc.vector.tensor_mul(a_i[:], a_i[:], a_r[:])
                nc.vector.tensor_sub(a_r[:], sq_r[:], sq_i[:])
                nc.vector.tensor_add(a_i[:], a_i[:], a_i[:])
            # B_coeff = (A-1) * conj(temp) / |temp|^2
            d2 = mr         # scratch
            nc.vector.tensor_mul(d2[:], tr[:], tr[:])
            nc.vector.tensor_mul(t_d[:], ti[:], ti[:])
            nc.vector.tensor_add(d2[:], d2[:], t_d[:])
            inv = mi        # scratch
            nc.vector.reciprocal(inv[:], d2[:])
            arm1 = sq_r
            nc.vector.tensor_scalar_add(arm1[:], a_r[:], -1.0)
            nc.vector.tensor_mul(bc_r[:], arm1[:], tr[:])
            nc.vector.tensor_mul(t_d[:], a_i[:], ti[:])
            nc.vector.tensor_add(bc_r[:], bc_r[:], t_d[:])
            nc.vector.tensor_mul(bc_r[:], bc_r[:], inv[:])
            nc.vector.tensor_mul(bc_i[:], a_i[:], tr[:])
            nc.vector.tensor_mul(t_d[:], arm1[:], ti[:])
            nc.vector.tensor_sub(bc_i[:], bc_i[:], t_d[:])
            nc.vector.tensor_mul(bc_i[:], bc_i[:], inv[:])

            # ---------------- conv / transform helpers ----------------
            def load_and_pad(src_ap, dst_pad, nch):
                """DRAM int8 flat (1024*nch,) -> dst_pad (nch, 1156) channel-major.

                The int8 payload is u / s_core; the dequant scale is folded
                into the B/D conv weights host-side."""
                u0 = work.tile([128, 8, nch], i8, tag="u0")
                nc.sync.dma_start(
                    u0[:], src_ap.rearrange("(t q u) -> q t u", q=128, u=nch))
                u0h = work.tile([128, 8, nch], f16, tag="u0h")
                nc.scalar.copy(u0h[:], u0[:])
                for t in range(8):
                    pt = pt_pool.tile([nch, 128], f16, tag="pt")
                    nc.tensor.transpose(pt[:], u0h[:, t, :], cst["ident16"][:])
                    pv = dst_pad.rearrange("c (r w) -> c r w", w=PW)
                    nc.scalar.copy(pv[:, 4 * t + 1:4 * t + 5, 1:33], pt[:])
                u2a = work.tile([128, NPAD], f16, tag="u2a")
                u2b = work.tile([128, NPAD], f16, tag="u2b")
                nc.gpsimd.tensor_copy(u2a[0:64, :], dst_pad[:])
                nc.gpsimd.tensor_copy(u2a[64:128, 0:NPAD - 1], dst_pad[:, 1:])
                nc.gpsimd.tensor_copy(u2b[0:64, :], dst_pad[:])
                nc.gpsimd.tensor_copy(u2b[64:128, 0:NPAD - 34], dst_pad[:, 34:])
                return u2a, u2b

            def conv_paired_into(psum_out, wp_tile, ws_tile, u2a, u2b, pad_tile,
                                 start, stop):
                """5-group paired conv accumulate: psum_out (cout, 512) x2 chunks."""
                va = u2a.rearrange("c (r w) -> c r w", w=PW)
                vb = u2b.rearrange("c (r w) -> c r w", w=PW)
                vs = pad_tile.rearrange("c (r w) -> c r w", w=PW)
                for c2 in range(2):
                    for g in range(4):
                        kh, kw = PAIR_OFFS[g] // PW, PAIR_OFFS[g] % PW
                        pv = va if PAIR_DELTA[g] == 1 else vb
                        nc.tensor.matmul(
                            psum_out[:, bass.ts(c2, 512)], wp_tile[:, g, :],
                            pv[:, kh + 16 * c2:kh + 16 * c2 + 16, kw:kw + 32],
                            start=(start and g == 0), stop=False)
                    nc.tensor.matmul(
                        psum_out[:, bass.ts(c2, 512)], ws_tile[:],
                        vs[:, 2 + 16 * c2:2 + 16 * c2 + 16, 2:34],
                        start=False, stop=stop)

            def fwd_stage(bu_ps):
                """bu_ps PSUM (128, 1024) -> (bhr, bhi) PSUM (128, 512) each."""
                s1 = work.tile([128, 1024], f32, tag="s1")
                nc.scalar.copy(s1[:, 0:512], bu_ps[:, 0:512])
                nc.scalar.copy(s1[:, 512:1024], bu_ps[:, 512:1024])
                t1 = work.tile([128, 8, 128], f32, tag="t1")
                for t in range(8):
                    pt = pt_pool.tile([128, 128], f32, tag="pt")
                    nc.tensor.transpose(pt[:], s1[:, bass.ts(t, 128)], cst["ident"][:])
                    nc.scalar.copy(t1[:, t, :], pt[:])
                rr = t1[:, :, 0:64]
                ri = t1[:, :, 64:128]
                yr = pw_pool.tile([128, 512], f32, tag="pw")
                yi = pw_pool.tile([128, 512], f32, tag="pw")
                nc.tensor.matmul(yr[:], cst["qwf_r"][:], rr, start=True, stop=False)
                nc.tensor.matmul(yr[:], cst["qwf_ni"][:], ri, start=False, stop=True)
                nc.tensor.matmul(yi[:], cst["qwf_i"][:], rr, start=True, stop=False)
                nc.tensor.matmul(yi[:], cst["qwf_r"][:], ri, start=False, stop=True)
                yw = work.tile([128, 8, 128], f32, tag="yw")
                nc.scalar.copy(yw[:, :, 0:64], yr[:].rearrange("p (t f) -> p t f", t=8))
                nc.scalar.copy(yw[:, :, 64:128], yi[:].rearrange("p (t f) -> p t f", t=8))
                z = work.tile([128, 1024], f32, tag="z")
                zv = z.rearrange("p (c tb dh) -> p c tb dh", tb=8, dh=4)
                for t in range(8):
                    pt = pt_pool.tile([128, 128], f32, tag="pt")
                    nc.tensor.transpose(pt[:], yw[:, t, :], cst["ident"][:])
                    nc.scalar.copy(zv[:, :, t, :],
                                   pt.rearrange("p (dh c) -> p c dh", dh=4))
                t2 = work.tile([128, 8, 128], f32, tag="t2")
                for uu in range(8):
                    pt = pt_pool.tile([128, 128], f32, tag="pt")
                    nc.tensor.transpose(pt[:], z[:, bass.ts(uu, 128)], cst["ident"][:])
                    nc.scalar.copy(t2[:, uu, :], pt[:])
                xr = t2[:, :, 0:64]
                xi = t2[:, :, 64:128]
                bhr = pbh_pool.tile([128, 512], f32, tag="pbh")
                bhi = pbh_pool.tile([128, 512], f32, tag="pbh")
                nc.tensor.matmul(bhr[:], cst["qhf_r"][:], xr, start=True, stop=False)
                nc.tensor.matmul(bhr[:], cst["qhf_ni"][:], xi, start=False, stop=True)
                nc.tensor.matmul(bhi[:], cst["qhf_i"][:], xr, start=True, stop=False)
                nc.tensor.matmul(bhi[:], cst["qhf_r"][:], xi, start=False, stop=True)
                return bhr, bhi

            def full_fwd(src_ap):
                u2a, u2b = load_and_pad(src_ap, upad, 64)
                bu = pacc.tile([128, 1024], f32, tag="pacc")
                conv_paired_into(bu, cst["wbp"], cst["wbs"], u2a, u2b, upad,
                                 start=True, stop=True)
                return fwd_stage(bu), u2a, u2b

            # ---- prologue: x0 ----
            (bhr0, bhi0), _, _ = full_fwd(seg8("x0", NPIX * U))
            q1 = tmp_pool.tile([128, 512], f32, tag="q1")
            q2 = tmp_pool.tile([128, 512], f32, tag="q2")
            nc.vector.tensor_mul(q1[:], bc_r[:], bhr0[:])
            nc.vector.tensor_mul(q2[:], bc_i[:], bhi0[:])
            nc.vector.tensor_sub(s_r[:], q1[:], q2[:])
            nc.vector.tensor_mul(q1[:], bc_r[:], bhi0[:])
            nc.vector.tensor_mul(q2[:], bc_i[:], bhr0[:])
            nc.vector.tensor_add(s_i[:], q1[:], q2[:])

            # ---- main loop ----
            for l in range(L):
                off_l = BLOB_OFF["useq"] + l * NPIX * U
                (bhr, bhi), u2a_l, u2b_l = full_fwd(
                    wire(off_l, NPIX * U))
                # scan update (DVE)
                t_a = tmp_pool.tile([128, 512], f32, tag="q1")
                t_b = tmp_pool.tile([128, 512], f32, tag="q2")
                t_c = tmp_pool.tile([128, 512], f32, tag="q3")
                t_dd = tmp_pool.tile([128, 512], f32, tag="q4")
                nr = tmp_pool.tile([128, 512], f32, tag="nr")
                nc.vector.tensor_mul(t_a[:], a_r[:], s_r[:])
                nc.vector.tensor_mul(t_b[:], a_i[:], s_i[:])
                nc.vector.tensor_sub(t_a[:], t_a[:], t_b[:])
                nc.vector.tensor_mul(t_c[:], bc_r[:], bhr[:])
                nc.vector.tensor_mul(t_dd[:], bc_i[:], bhi[:])
                nc.vector.tensor_sub(t_c[:], t_c[:], t_dd[:])
                nc.vector.tensor_add(nr[:], t_a[:], t_c[:])
                nc.vector.tensor_mul(t_a[:], a_r[:], s_i[:])
                nc.vector.tensor_mul(t_b[:], a_i[:], s_r[:])
                nc.vector.tensor_add(t_a[:], t_a[:], t_b[:])
                nc.vector.tensor_mul(t_c[:], bc_r[:], bhi[:])
                nc.vector.tensor_mul(t_dd[:], bc_i[:], bhr[:])
                nc.vector.tensor_add(t_c[:], t_c[:], t_dd[:])
                nc.vector.tensor_add(s_i[:], t_a[:], t_c[:])
                nc.vector.tensor_copy(s_r[:], nr[:])

                # inverse transform
                x1r = pw_pool.tile([128, 512], f32, tag="pw")
                x1i = pw_pool.tile([128, 512], f32, tag="pw")
                nc.tensor.matmul(x1r[:], cst["qhi_r"][:], s_r[:], start=True, stop=False)
                nc.tensor.matmul(x1r[:], cst["qhi_ni"][:], s_i[:], start=False, stop=True)
                nc.tensor.matmul(x1i[:], cst["qhi_i"][:], s_r[:], start=True, stop=False)
                nc.tensor.matmul(x1i[:], cst["qhi_r"][:], s_i[:], start=False, stop=True)
                xs1 = work.tile([128, 8, 128], f32, tag="xs1")
                nc.scalar.copy(xs1[:, :, 0:64], x1r[:].rearrange("p (t f) -> p t f", t=8))
                nc.scalar.copy(xs1[:, :, 64:128], x1i[:].rearrange("p (t f) -> p t f", t=8))
                zi = work.tile([128, 1024], f32, tag="zi")
                ziv = zi.rearrange("p (h ub dc) -> p h ub dc", ub=8, dc=4)
                for uu in range(8):
                    pt = pt_pool.tile([128, 128], f32, tag="pt")
                    nc.tensor.transpose(pt[:], xs1[:, uu, :], cst["ident"][:])
                    nc.scalar.copy(ziv[:, :, uu, :],
                                   pt.rearrange("p (dc h) -> p h dc", dc=4))
                t2i = work.tile([128, 8, 128], f32, tag="t2i")
                for vv in range(8):
                    pt = pt_pool.tile([128, 128], f32, tag="pt")
                    nc.tensor.transpose(pt[:], zi[:, bass.ts(vv, 128)], cst["ident"][:])
                    nc.scalar.copy(t2i[:, vv, :], pt[:])
                wr = t2i[:, :, 0:64]
                wi = t2i[:, :, 64:128]
                xspr = pw_pool.tile([128, 512], f32, tag="pw")
                xspi = pw_pool.tile([128, 512], f32, tag="pw")
                nc.tensor.matmul(xspr[:], cst["qwi_r"][:], wr, start=True, stop=False)
                nc.tensor.matmul(xspr[:], cst["qwi_ni"][:], wi, start=False, stop=True)
                nc.tensor.matmul(xspi[:], cst["qwi_i"][:], wr, start=True, stop=False)
                nc.tensor.matmul(xspi[:], cst["qwi_r"][:], wi, start=False, stop=True)
                xsp = work.tile([128, 8, 128], f32, tag="xsp")
                nc.scalar.copy(xsp[:, :, 0:64], xspr[:].rearrange("p (t f) -> p t f", t=8))
                nc.scalar.copy(xsp[:, :, 64:128], xspi[:].rearrange("p (t f) -> p t f", t=8))
                for vv in range(8):
                    pt = pt_pool.tile([128, 128], f32, tag="pt")
                    nc.tensor.transpose(
                        pt[:], xsp[:, vv, :], cst["ident"][:])
                    cv = cpad.rearrange("c (r w) -> c r w", w=PW)
                    nc.scalar.copy(cv[:, 4 * vv + 1:4 * vv + 5, 1:33], pt[:])
                # C conv + D conv into one PSUM, then gelu
                yps = pacc.tile([64, 1024], f32, tag="pacc")
                cpv = cpad.rearrange("c (r w) -> c r w", w=PW)
                for c2 in range(2):
                    for tidx, (kh, kw) in enumerate(taps):
                        nc.tensor.matmul(
                            yps[:, bass.ts(c2, 512)], cst["wc"][:, tidx, :],
                            cpv[:, kh + 16 * c2:kh + 16 * c2 + 16, kw:kw + 32],
                            start=(tidx == 0), stop=False)
                conv_paired_into(yps, cst["wdp"], cst["wds"], u2a_l, u2b_l, upad,
                                 start=False, stop=True)
                yout = work.tile([64, 1024], f32, tag="yout")
                nc.scalar.activation(yout[:], yps[:], AF.Gelu_apprx_tanh)
                # per-l int8 quantization scale: qs = 127 / max|yout|
                nc.vector.tensor_reduce(rmax[0:64, :], yout[:],
                                        axis=mybir.AxisListType.X,
                                        op=mybir.AluOpType.max,
                                        apply_absolute_value=True)
                gall = tmp_pool.tile([128, 1], f32, tag="gall")
                nc.gpsimd.partition_all_reduce(
                    gall[:], rmax[:], channels=128,
                    reduce_op=bass_isa.ReduceOp.max)
                nc.vector.tensor_scalar_max(gall[:], gall[:], 1e-6)
                q1s = tmp_pool.tile([128, 1], f32, tag="q1s")
                nc.vector.tensor_scalar_mul(q1s[:], gall[:], 1.0 / 127.0)
                nc.scalar.copy(ds_all[:, l:l + 1], q1s[0:1, :])
                qsb = tmp_pool.tile([128, 1], f32, tag="qsb")
                nc.vector.reciprocal(qsb[:], q1s[:])
                osb = work.tile([128, 8, 64], i8, tag="osb")
                for t in range(8):
                    pt = pt_pool.tile([128, 64], f32, tag="pt")
                    nc.tensor.transpose(
                        pt[:], yout[:, bass.ts(t, 128)], cst["ident"][:64, :64])
                    nc.scalar.activation(osb[:, t, :], pt[:], AF.Copy,
                                         scale=qsb[:])
                ytgt = ya_d[l] if l < L // 2 else yb_d[l - L // 2]
                nc.sync.dma_start(
                    ytgt.rearrange("(t q) u -> q t u", q=128), osb[:])
            nc.sync.dma_start(ysc_d[:], ds_all[:])
    nc.finalize()
    return nc


# ----------------------------------------------------------------------------
# Custom cached runner (bass_exec via PJRT, no retrace, no zero buffers)
# ----------------------------------------------------------------------------

_CACHE = {}


def _get_runner():
    if "fn" in _CACHE:
        return _CACHE["fn"]
    import jax
    import concourse.mybir as mybir
    from concourse import bass2jax
    from jax.sharding import Mesh, PartitionSpec
    from jax.experimental.shard_map import shard_map

    nc = build_bass()
    assert nc.dbg_addr is None
    bass2jax.install_neuronx_cc_hook()

    partition_name = nc.partition_id_tensor.name if nc.partition_id_tensor else None
    in_names, out_names, out_avals = [], [], []
    for alloc in nc.m.functions[0].allocations:
        if not isinstance(alloc, mybir.MemoryLocationSet):
            continue
        name = alloc.memorylocations[0].name
        if alloc.kind == "ExternalInput":
            if name != partition_name:
                in_names.append(name)
        elif alloc.kind == "ExternalOutput":
            out_names.append(name)
            out_avals.append(jax.core.ShapedArray(tuple(alloc.tensor_shape),
                                                  mybir.dt.np(alloc.dtype)))
    user_in_names = list(in_names)
    if partition_name is not None:
        in_names.append(partition_name)

    def _body(*args):
        operands = list(args)
        if partition_name is not None:
            operands.append(bass2jax.partition_id_tensor())
        outs = bass2jax._bass_exec_p.bind(
            *operands,
            out_avals=tuple(out_avals),
            in_names=tuple(in_names),
            out_names=tuple(out_names),
            lowering_input_output_aliases=(),
            sim_require_finite=True,
            sim_require_nnan=True,
            nc=nc,
        )
        return tuple(outs)

    devices = jax.devices()[:BSZ]
    mesh = Mesh(np.asarray(devices), ("core",))
    in_avals = []
    for alloc in nc.m.functions[0].allocations:
        if not isinstance(alloc, mybir.MemoryLocationSet):
            continue
        name = alloc.memorylocations[0].name
        if alloc.kind == "ExternalInput" and name in user_in_names:
            shp = list(alloc.tensor_shape)
            shp[0] *= BSZ
            in_avals.append(
                jax.ShapeDtypeStruct(tuple(shp), mybir.dt.np(alloc.dtype)))

    def _compile():
        return jax.jit(
            shard_map(_body, mesh=mesh,
                      in_specs=(PartitionSpec("core"),) * len(user_in_names),
                      out_specs=(PartitionSpec("core"),) * len(out_names),
                      check_rep=False),
            keep_unused=True,
        ).lower(*in_avals).compile()

    try:
        fn = bass2jax.fast_dispatch_compile(_compile)
    except Exception:
        fn = jax.jit(
            shard_map(_body, mesh=mesh,
                      in_specs=(PartitionSpec("core"),) * len(user_in_names),
                      out_specs=(PartitionSpec("core"),) * len(out_names),
                      check_rep=False),
            keep_unused=True,
        )
    from jax.sharding import NamedSharding
    _CACHE["devices"] = devices
    _CACHE["sharding"] = NamedSharding(mesh, PartitionSpec("core"))
    _CACHE["fn"] = (fn, user_in_names, out_names)
    return _CACHE["fn"]


def kernel(**inputs):
    import jax
    from jax.sharding import NamedSharding, PartitionSpec

    fn, user_in_names, out_names = _get_runner()
    inputs = {k: np.asarray(v) for k, v in inputs.items()}

    dyn = pack_dynamic(
        inputs["Lambda_re"], inputs["Lambda_im"], inputs["values"],
        inputs["log_step"], inputs["B_r"], inputs["B_i"], inputs["C_r"],
        inputs["C_i"], inputs["D_kernel"])
    w32 = {k: dyn[k].astype(np.float32).reshape(-1)
           for k in ("wbp", "wdp", "wbs", "wds")}
    wc16 = dyn["wc"].reshape(-1)
    pv32 = dyn["pvec"].reshape(-1)

    # useq/x0 ship as int8 (per-core scale folded into the B/D conv weight
    # copies); each core's wire blob is packed and device_put independently
    # so packing overlaps the serial axon uploads.
    useq = inputs["input_sequence"].reshape(L, BSZ, NPIX, U)
    x0 = inputs["x0"].reshape(BSZ, NPIX, U)
    devices = _CACHE["devices"]
    sharding = _CACHE["sharding"]

    # reuse per-core wire buffers and f32 scratch across calls: avoids ~20MB
    # of fresh page-faulted allocations inside the timed call
    if "blob_bufs" not in _CACHE:
        _CACHE["blob_bufs"] = [np.empty(NWIRE, dtype=np.int8)
                               for _ in range(BSZ)]
        _CACHE["scratch"] = [np.empty((L, NPIX, U), dtype=np.float32)
                             for _ in range(BSZ)]

    def _pack_core(b):
        ub, xb = useq[:, b], x0[b]
        s = np.float32(max(np.abs(ub).max(), np.abs(xb).max()) / 127.0)
        blob = _CACHE["blob_bufs"][b]
        tmp = _CACHE["scratch"][b]
        o, n = BLOB_OFF["useq"], L * NPIX * U
        np.multiply(ub, np.float32(1.0) / s, out=tmp)
        np.rint(tmp, out=tmp)
        np.copyto(blob[o:o + n].reshape(L, NPIX, U), tmp, casting="unsafe")
        o, n = BLOB_OFF["x0"], NPIX * U
        np.copyto(blob[o:o + n].reshape(NPIX, U),
                  np.rint(xb * (np.float32(1.0) / s)), casting="unsafe")
        for k, v in w32.items():
            o = BLOB_OFF[k]
            np.copyto(blob[o:o + 2 * v.size].view(np.float16), v * s,
                      casting="unsafe")
        o = BLOB_OFF["wc"]
        blob[o:o + 2 * wc16.size].view(np.float16)[:] = wc16
        o = BLOB_OFF["pvec"]
        blob[o:o + 4 * pv32.size].view(np.float32)[:] = pv32
        return (jax.device_put(blob[:NSPLIT], devices[b]),
                jax.device_put(blob[NSPLIT:], devices[b]))

    NSPLIT = (L // 2) * NPIX * U
    from concurrent.futures import ThreadPoolExecutor
    with ThreadPoolExecutor(max_workers=8) as ex:
        shards = list(ex.map(_pack_core, range(BSZ)))
    garr_a = jax.make_array_from_single_device_arrays(
        (BSZ * NSPLIT,), sharding, [s[0] for s in shards])
    garr_b = jax.make_array_from_single_device_arrays(
        (BSZ * (NWIRE - NSPLIT),), sharding, [s[1] for s in shards])
    args = {"bloba": garr_a, "blobb": garr_b}
    args = [args[name] for name in user_in_names]
    try:
        outs = fn(*args)
    except Exception:
        # transient device hiccups (e.g. NRT exec-unit recovery) — retry once
        import time as _time
        _time.sleep(2.0)
        outs = fn(*args)
    ya_arr = outs[out_names.index("ya")]
    yb_arr = outs[out_names.index("yb")]
    ysc_arr = outs[out_names.index("ysc")]

    # fetch + dequantize shards in parallel: concurrent D2H requests keep
    # the axon pipe busy, each shard is processed as soon as its device is
    # done, and the int8->f32 multiplies run in the threads
    try:
        ya_arr.copy_to_host_async()
        yb_arr.copy_to_host_async()
        ysc_arr.copy_to_host_async()
    except Exception:
        pass
    out = np.empty((L, BSZ, NPIX, U), dtype=np.float32)
    LH = L // 2

    sc_by_b = {(s.index[0].start or 0): s for s in ysc_arr.addressable_shards}

    def _fetch_sc(b):
        return b, np.asarray(sc_by_b[b].data)[0]            # (L,) f32

    def _fetch_dequant(job):
        yshard, l0 = job
        b = yshard.index[0].start // LH if yshard.index[0].start else 0
        sc = sc_host[b]
        data = np.asarray(yshard.data)                      # (LH, NPIX, U) int8
        np.multiply(data, sc[l0:l0 + LH].reshape(LH, 1, 1),
                    out=out[l0:l0 + LH, b], casting="unsafe")

    jobs = [(s, 0) for s in ya_arr.addressable_shards] + \
           [(s, LH) for s in yb_arr.addressable_shards]
    with ThreadPoolExecutor(max_workers=16) as ex:
        sc_host = dict(ex.map(_fetch_sc, range(BSZ)))
        list(ex.map(_fetch_dequant, jobs))
    return out.reshape(L, BSZ, H, W, U)


# ----------------------------------------------------------------------------
# Numpy mock of the device pipeline (for layout validation): run with
# `python test.py mock`. Mirrors the device ops in f32.
# ----------------------------------------------------------------------------

def host_constants_mock(Lambda_re, Lambda_im, values, log_step, B_r, B_i,
                        C_r, C_i, D_kernel):
    scst = static_constants()
    dyn = pack_dynamic(Lambda_re, Lambda_im, values, log_step, B_r, B_i,
                       C_r, C_i, D_kernel)
    ar, ai, bcr, bci = abc_host(dyn["pvec"].astype(np.float64), scst)
    cst = dict(scst)
    cst.update({k: v.astype(np.float32) for k, v in dyn.items()})
    cst.update({"a_r": ar, "a_i": ai, "bc_r": bcr, "bc_i": bci})
    return cst


def _mock_core(useq, x0, cst):
    """useq (L, 1024, 64), x0 (1024, 64) -> y (L, 1024, 64). Mirrors device ops."""
    taps = [(kh, kw) for kh in range(3) for kw in range(3)]

    def pad_cm(img_cm):  # (ch, 1024) -> (ch, 34*34) zero border
        nch = img_cm.shape[0]
        p = np.zeros((nch, PW, PW), dtype=np.float32)
        p[:, 1:33, 1:33] = img_cm.reshape(nch, 32, 32)
        return p.reshape(nch, NPAD)

    def conv_cm(pad, wk):  # pad (cin, 1156), wk (cin, 9, cout) -> (cout, 1024)
        acc = np.zeros((wk.shape[2], NPIX), dtype=np.float32)
        for t, (kh, kw) in enumerate(taps):
            win = pad.reshape(-1, PW, PW)[:, kh:kh + 32, kw:kw + 32].reshape(-1, NPIX)
            acc += wk[:, t, :].astype(np.float32).T @ win
        return acc

    def win_of(buf, off):  # buf (nch, 1156) -> strided window (nch, 1024)
        v = np.zeros((buf.shape[0], 16 * 2, 32), dtype=np.float32)
        for c2 in range(2):
            for r in range(16):
                s = off + (16 * c2 + r) * PW
                v[:, 16 * c2 + r, :] = buf[:, s:s + 32]
        return v.reshape(buf.shape[0], NPIX)

    def conv_paired(up2d1, up2d34, upad_, wp, ws):
        acc = np.zeros((wp.shape[2], NPIX), dtype=np.float32)
        for g in range(4):
            buf = up2d1 if PAIR_DELTA[g] == 1 else up2d34
            acc += wp[:, g, :].astype(np.float32).T @ win_of(buf, PAIR_OFFS[g])
        acc += ws.astype(np.float32).T @ win_of(upad_, SOLO_OFF)
        return acc

    def fwd_transform(bu_cm):  # (128=[r|i]p, 1024 pix) -> bhr, bhi (128, 512)
        t1 = np.zeros((128, 8, 128), dtype=np.float32)
        for t in range(8):
            t1[:, t, :] = bu_cm[:, t * 128:(t + 1) * 128].T
        rr = t1[:, :, 0:64].reshape(128, 512)
        ri = t1[:, :, 64:128].reshape(128, 512)
        yr = cst["qwf_r"].T @ rr + cst["qwf_ni"].T @ ri
        yi = cst["qwf_i"].T @ rr + cst["qwf_r"].T @ ri
        y = np.zeros((128, 8, 2, 64), dtype=np.float32)
        y[:, :, 0, :] = yr.reshape(128, 8, 64)
        y[:, :, 1, :] = yi.reshape(128, 8, 64)
        z2 = np.zeros((128, 32, 8, 4), dtype=np.float32)   # (ch, c, hb, dh)
        for t in range(8):
            pt = y[:, t, :, :].reshape(128, 128).T         # (ch, (dh, c))
            z2[:, :, t, :] = pt.reshape(128, 4, 32).transpose(0, 2, 1)
        z2 = z2.reshape(128, 1024)
        t2 = np.zeros((128, 8, 128), dtype=np.float32)
        for uu in range(8):
            t2[:, uu, :] = z2[:, uu * 128:(uu + 1) * 128].T
        xr = t2[:, :, 0:64].reshape(128, 512)
        xi = t2[:, :, 64:128].reshape(128, 512)
        bhr = cst["qhf_r"].T @ xr + cst["qhf_ni"].T @ xi
        bhi = cst["qhf_i"].T @ xr + cst["qhf_r"].T @ xi
        return bhr, bhi

    def inv_transform(sr, si):  # scan state (128,512) -> xsp (128=[r|i]p, 1024 pix)
        x1r = cst["qhi_r"].T @ sr + cst["qhi_ni"].T @ si
        x1i = cst["qhi_i"].T @ sr + cst["qhi_r"].T @ si
        xs1 = np.zeros((128, 8, 2, 64), dtype=np.float32)
        xs1[:, :, 0, :] = x1r.reshape(128, 8, 64)
        xs1[:, :, 1, :] = x1i.reshape(128, 8, 64)
        z2i = np.zeros((128, 32, 8, 4), dtype=np.float32)  # (ch, h, ub, dc)
        for uu in range(8):
            pt = xs1[:, uu, :, :].reshape(128, 128).T      # (ch, (dc, h))
            z2i[:, :, uu, :] = pt.reshape(128, 4, 32).transpose(0, 2, 1)
        z2i = z2i.reshape(128, 1024)
        t2i = np.zeros((128, 8, 128), dtype=np.float32)
        for vv in range(8):
            t2i[:, vv, :] = z2i[:, vv * 128:(vv + 1) * 128].T
        wr = t2i[:, :, 0:64].reshape(128, 512)
        wi = t2i[:, :, 64:128].reshape(128, 512)
        xspr = cst["qwi_r"].T @ wr + cst["qwi_ni"].T @ wi
        xspi = cst["qwi_i"].T @ wr + cst["qwi_r"].T @ wi
        xsp = np.zeros((128, 8, 2, 64), dtype=np.float32)
        xsp[:, :, 0, :] = xspr.reshape(128, 8, 64)
        xsp[:, :, 1, :] = xspi.reshape(128, 8, 64)
        out = np.zeros((128, 1024), dtype=np.float32)      # (ch=[r|i]p, pix)
        for vv in range(8):
            out[:, vv * 128:(vv + 1) * 128] = xsp[:, vv, :].reshape(128, 128).T
        return out

    def fwd_from_img(img):  # (1024, 64) -> bhr, bhi
        up = pad_cm(img.T.astype(np.float32))
        up2d1 = np.zeros((128, NPAD), dtype=np.float32)
        up2d1[0:64] = up
        up2d1[64:128, 0:NPAD - 1] = up[:, 1:]
        up2d34 = np.zeros((128, NPAD), dtype=np.float32)
        up2d34[0:64] = up
        up2d34[64:128, 0:NPAD - 34] = up[:, 34:]
        bu = conv_paired(up2d1, up2d34, up, cst["wbp"], cst["wbs"])
        return fwd_transform(bu), (up, up2d1, up2d34)

    y_out = np.zeros((L, NPIX, U), dtype=np.float32)
    (bhr0, bhi0), _ = fwd_from_img(x0)
    sr = cst["bc_r"] * bhr0 - cst["bc_i"] * bhi0
    si = cst["bc_r"] * bhi0 + cst["bc_i"] * bhr0
    for l in range(L):
        (bhr, bhi), upad = fwd_from_img(useq[l])
        nsr = (cst["a_r"] * sr - cst["a_i"] * si) + (cst["bc_r"] * bhr - cst["bc_i"] * bhi)
        nsi = (cst["a_r"] * si + cst["a_i"] * sr) + (cst["bc_r"] * bhi + cst["bc_i"] * bhr)
        sr, si = nsr, nsi
        xsp = inv_transform(sr, si)                        # (128, 1024)
        cpad = pad_cm(xsp)                                 # (128, 1156)
        up_, u2d1_, u2d34_ = upad
        yacc = conv_cm(cpad, cst["wc"]) + conv_paired(u2d1_, u2d34_, up_, cst["wdp"], cst["wds"])
        g = 0.5 * yacc * (1.0 + np.tanh(0.7978845608028654 * (yacc + 0.044715 * yacc ** 3)))
        y_out[l] = g.T
    return y_out


def mock_kernel(**inputs):
    cst = host_constants_mock(
        inputs["Lambda_re"], inputs["Lambda_im"], inputs["values"], inputs["log_step"],
        inputs["B_r"], inputs["B_i"], inputs["C_r"], inputs["C_i"], inputs["D_kernel"])
    useq = inputs["input_sequence"].reshape(L, BSZ, NPIX, U)
    x0 = inputs["x0"].reshape(BSZ, NPIX, U)
    outs = [_mock_core(useq[:, b], x0[b], cst) for b in range(BSZ)]
    return np.stack(outs, axis=1).reshape(L, BSZ, H, W, U)


# revision 56
# speedup vs baseline: 1.0391x; 1.0391x over previous
"""Trainium2 Bass kernel for the BTST-SSM problem.

Math: 2D state-space model. Per l: u -> conv(B) -> DST-eigendomain ->
diagonal linear recurrence over l -> inverse DST -> conv(C) + conv(D) -> gelu.

Sharding: batch (8) across 8 cores, one sample per core. No collectives.

The warm call is axon-tunnel-bandwidth bound (~30-40 MB/s, half duplex), so
the design minimizes wire bytes and per-RPC overheads:
  - useq/x0 ship as int8 (per-core scale folded into that core's B/D conv
    weight copies; dequant on device is a free int8->fp16 copy)
  - conv weights ship as fp16; y returns as int8 with per-l scales computed
    on device (vector reduce_max + gpsimd partition_all_reduce)
  - all per-core inputs ride in ONE int8 wire tensor (fp16/f32 segments via
    AP.bitcast), packed per core in threads that overlap the serial uploads
  - A_bar/B_coeff are computed on device from tiny per-channel vectors:
    A^(1/32) via Exp/Sin (args stay in the accurate range) then 5 complex
    squarings; B = (A-1)*conj(temp)/|temp|^2 with vector.reciprocal
  - input-independent DST/basis constants are baked into the NEFF via
    inline_tensor (shipped once at executable load)
  - custom cached-jit runner built directly on _bass_exec_p: no per-call
    retrace, no zero donation buffers, fast-dispatch compile
  - y shards are fetched + dequantized concurrently (8 threads)

Per-core layouts (SBUF tiles are (128 partitions, free)):
  - channel-major image: (ch, h*w) padded to (ch, 34*34) for SAME conv
  - spatial tiles for transforms: partition = (g4, x32) block-diag groups
  - scan state: partition q = dc*32 + a  (c = 4u + dc), free col = u*64 + p
All transform matrices are packed host-side as kron(I4, blk) lhsT tensors;
complex arithmetic is done with +/- weight copies accumulated in PSUM.

Accuracy: ~1.43e-2 max-relative (dominated by int8 input quantization,
validated against the reference in the numpy mock; deterministic).
"""

import os
import sys
import numpy as np

sys.path.insert(0, "/opt/trn_rl_repo")

H = W = 32
P = 64
U = 64
L = 32
BSZ = 8
PW = 34          # padded width/height
NPIX = H * W     # 1024
NPAD = PW * PW   # 1156
PI = float(np.pi)


# ----------------------------------------------------------------------------
# Host constant computation (float64 -> float32)
# ----------------------------------------------------------------------------

def _dst_q(n):
    idx = np.arange(1, n + 1, dtype=np.float64)
    s = np.sin(np.pi * idx[:, None] * idx[None, :] / (n + 1)) / np.sqrt((n + 1) / 2.0)
    phase = np.exp(1j * (np.pi / 2.0) * idx)
    return phase[:, None] * s


def _softmax(x, axis):
    m = np.max(x, axis=axis, keepdims=True)
    e = np.exp(x - m)
    return e / np.sum(e, axis=axis, keepdims=True)


def static_constants():
    """Input-independent constants baked into the NEFF as inline tensors."""
    Qh = _dst_q(H)
    Qw = _dst_q(W)
    Qh_inv = np.conj(Qh).T
    Qw_inv = np.conj(Qw).T

    def kr(m):
        return np.kron(np.eye(4), m).astype(np.float32)

    ih = np.arange(1, H + 1, dtype=np.float64)
    iw = np.arange(1, W + 1, dtype=np.float64)
    ch = 2.0 * np.cos(np.pi * ih / (H + 1))
    cw = 2.0 * np.cos(np.pi * iw / (W + 1))
    # packed (q=(dc,a), col=(u,p)) grids of cw[c]=cw[4u+dc] and ch[a]
    cw_pk = np.zeros((128, 512), dtype=np.float64)
    ch_pk = np.zeros((128, 512), dtype=np.float64)
    for dc in range(4):
        for uu in range(8):
            cw_pk[dc * 32:(dc + 1) * 32, uu * 64:(uu + 1) * 64] = cw[4 * uu + dc]
    for a in range(32):
        ch_pk[a::32, :] = ch[a]

    return {
        "qwf_r": kr(np.real(Qw_inv).T), "qwf_i": kr(np.imag(Qw_inv).T),
        "qwf_ni": kr(-np.imag(Qw_inv).T),
        "qhf_r": kr(np.real(Qh_inv).T), "qhf_i": kr(np.imag(Qh_inv).T),
        "qhf_ni": kr(-np.imag(Qh_inv).T),
        "qhi_r": kr(np.real(Qh).T), "qhi_i": kr(np.imag(Qh).T),
        "qhi_ni": kr(-np.imag(Qh).T),
        "qwi_r": kr(np.real(Qw).T), "qwi_i": kr(np.imag(Qw).T),
        "qwi_ni": kr(-np.imag(Qw).T),
        "ident": np.eye(128, dtype=np.float32),
        "ident16": np.eye(128, dtype=np.float16),
        "ones1": np.ones((1, 128), dtype=np.float32),
        "cw_pk": cw_pk.astype(np.float32),
        "ch_pk": ch_pk.astype(np.float32),
    }


PAIRS = [((0, 0), (0, 1)), ((1, 0), (1, 1)), ((2, 0), (2, 1)), ((0, 2), (1, 2))]
PAIR_OFFS = [0, 34, 68, 2]      # window offset of t0 per pair group
PAIR_DELTA = [1, 1, 1, 34]      # o(t1) - o(t0); delta 1 -> up2d1, 34 -> up2d34
SOLO_OFF = 70                   # (2,2)

# single int8 wire blob per core: quantized useq+x0 (per-core scale folded
# into that core's B/D conv weights), fp16 conv weights, and f32 pvec — one
# large axon transfer instead of many. Offsets are in BYTES, 64-aligned.
_SEGS8 = [("useq", L * NPIX * U), ("x0", NPIX * U)]          # int8, 1B each
_SEGS16 = [("wbp", 128 * 4 * 128), ("wdp", 128 * 4 * 64), ("wbs", 64 * 128),
           ("wds", 64 * 64), ("wc", 128 * 9 * 64)]           # fp16, 2B each
BLOB_OFF = {}
_o = 0
for _n, _s in _SEGS8:
    BLOB_OFF[_n] = _o
    _o += _s
for _n, _s in _SEGS16:
    BLOB_OFF[_n] = _o
    _o += 2 * _s
BLOB_OFF["pvec"] = _o
_o += 6 * 512 * 4
NWIRE = _o


def pack_dynamic(Lambda_re, Lambda_im, values, log_step, B_r, B_i, C_r, C_i,
                 D_kernel):
    """Per-call small tensors: channel vectors + fp16-packed conv weights."""
    lam_re = np.minimum(Lambda_re.astype(np.float64), -1e-4)
    lam_im = Lambda_im.astype(np.float64)
    step = np.exp(log_step.astype(np.float64))
    v = _softmax(values.astype(np.float64), axis=-1) * 4.0
    xk, yk, zk, wk = v[:, 0], v[:, 1], v[:, 2], v[:, 3]
    kv = np.stack(((xk + yk - 2) / 4, (xk + zk - 2) / 4, (xk + wk - 2) / 8),
                  axis=-1)                                     # (P, 3)
    pvec = np.zeros((6, 512), dtype=np.float64)
    rows = [lam_re, lam_im, step, kv[:, 0], kv[:, 1], kv[:, 2]]
    for r, vec in enumerate(rows):
        pvec[r] = np.tile(vec, 8)
    pvec = pvec.astype(np.float32)

    wb = np.concatenate([B_r, B_i], axis=-1).transpose(2, 0, 1, 3) \
        .reshape(U, 9, 2 * P)
    wc = np.concatenate([2.0 * C_r, -2.0 * C_i], axis=2).transpose(2, 0, 1, 3) \
        .reshape(2 * P, 9, U).astype(np.float16)
    wd = D_kernel.transpose(2, 0, 1, 3).reshape(U, 9, U)
    wb9 = wb.reshape(U, 3, 3, 2 * P)
    wd9 = wd.reshape(U, 3, 3, U)

    def pack_pairs(w9, cout):
        out = np.zeros((128, 4, cout), dtype=np.float16)
        for g, (t0, t1) in enumerate(PAIRS):
            out[0:64, g, :] = w9[:, t0[0], t0[1], :]
            out[64:128, g, :] = w9[:, t1[0], t1[1], :]
        return out

    return {
        "pvec": pvec,
        "wbp": pack_pairs(wb9, 2 * P),
        "wdp": pack_pairs(wd9, U),
        "wbs": np.ascontiguousarray(wb9[:, 2, 2, :]).astype(np.float16),
        "wds": np.ascontiguousarray(wd9[:, 2, 2, :]).astype(np.float16),
        "wc": wc,
    }


# ----------------------------------------------------------------------------
# Host reference of the device A_bar/B_coeff computation (for validation)
# ----------------------------------------------------------------------------

def abc_host(pvec, cst):
    """Mirror of the device prologue, in numpy f32."""
    lam_re = pvec[0][None, :]      # broadcast rows (128, 512)
    lam_im = pvec[1][None, :]
    step = pvec[2][None, :]
    kv0, kv1, kv2 = pvec[3][None, :], pvec[4][None, :], pvec[5][None, :]
    CW, CH = cst["cw_pk"], cst["ch_pk"]
    D = (kv2 * CW + kv1) * CH + kv0 * CW + 1.0
    tr = lam_re * D
    ti = lam_im * D
    mr = tr * step
    mi = ti * step
    er32 = np.exp(mr / 32.0)
    s32 = np.sin(mi / 32.0)
    s64 = np.sin(mi / 64.0)
    c32 = 1.0 - 2.0 * s64 * s64
    ar, ai = er32 * c32, er32 * s32
    for _ in range(5):
        ar, ai = ar * ar - ai * ai, 2.0 * ar * ai
    d2 = tr * tr + ti * ti
    inv = 1.0 / d2
    arm1 = ar - 1.0
    bc_r = (arm1 * tr + ai * ti) * inv
    bc_i = (ai * tr - arm1 * ti) * inv
    return ar, ai, bc_r, bc_i


# ----------------------------------------------------------------------------
# Bass kernel
# ----------------------------------------------------------------------------

def build_bass():
    import concourse.bass as bass
    import concourse.bacc as bacc
    import concourse.mybir as mybir
    import concourse.tile as tile
    import concourse.bass_isa as bass_isa

    f32 = mybir.dt.float32
    f16 = mybir.dt.float16
    AF = mybir.ActivationFunctionType
    nc = bacc.Bacc(None)

    i8 = mybir.dt.int8
    # wire blob splits into two input tensors per core so each core exposes
    # two H2D buffers (more in-flight upload RPCs on the axon pipe)
    NSPLIT = (L // 2) * NPIX * U        # first half of useq
    bloba_d = nc.dram_tensor("bloba", [NSPLIT], i8, kind="ExternalInput")
    blobb_d = nc.dram_tensor("blobb", [NWIRE - NSPLIT], i8,
                             kind="ExternalInput")

    def wire(off, size):
        if off + size <= NSPLIT:
            return bloba_d[off:off + size]
        assert off >= NSPLIT
        return blobb_d[off - NSPLIT:off - NSPLIT + size]

    def seg8(name, size):
        return wire(BLOB_OFF[name], size)

    def seg16(name, size):
        return wire(BLOB_OFF[name], 2 * size).bitcast(f16)

    def seg32(name, size):
        return wire(BLOB_OFF[name], 4 * size).bitcast(f32)

    dyn_shapes = {
        "wbp": (128, 4, 128), "wdp": (128, 4, 64),
        "wbs": (64, 128), "wds": (64, 64),
        "wc": (128, 9, 64),
    }
    # y splits into four output tensors so each core exposes four D2H
    # buffers (more in-flight fetch RPCs on the axon pipe)
    yq_d = [nc.dram_tensor(f"y{i}", [L // 4, NPIX, U], i8,
                           kind="ExternalOutput") for i in range(4)]
    ysc_d = nc.dram_tensor("ysc", [1, L], f32, kind="ExternalOutput")

    scst = static_constants()
    cst_dram = {k: nc.inline_tensor(v, name=k) for k, v in scst.items()}

    taps = [(kh, kw) for kh in range(3) for kw in range(3)]

    with tile.TileContext(nc) as tc:
        with (
            tc.tile_pool(name="cpool", bufs=1) as cpool,
            tc.tile_pool(name="state", bufs=1) as spool,
            tc.tile_pool(name="work", bufs=2) as work,
            tc.tile_pool(name="tmp", bufs=2) as tmp_pool,
            tc.tile_pool(name="pacc", bufs=1, space="PSUM") as pacc,
            tc.tile_pool(name="pt", bufs=2, space="PSUM") as pt_pool,
            tc.tile_pool(name="pw", bufs=2, space="PSUM") as pw_pool,
            tc.tile_pool(name="pbh", bufs=2, space="PSUM") as pbh_pool,
        ):
            cst = {}
            for k, v in scst.items():
                t = cpool.tile(list(v.shape), f16 if v.dtype == np.float16 else f32,
                               tag=k, name=k)
                nc.sync.dma_start(t[:], cst_dram[k][:])
                cst[k] = t
            for k, shp in dyn_shapes.items():
                t = cpool.tile(list(shp), f16, tag=k, name=k)
                nelem = int(np.prod(shp))
                src = seg16(k, nelem)
                if len(shp) == 2:
                    src = src.rearrange("(p a) -> p a", p=shp[0])
                else:
                    src = src.rearrange("(p a b) -> p a b", p=shp[0], a=shp[1])
                nc.sync.dma_start(t[:], src)
                cst[k] = t
            pvec_all = seg32("pvec", 6 * 512)
            pvec_rows = []
            for r in range(6):
                rt = cpool.tile([1, 512], f32, tag=f"pvec{r}", name=f"pvec_t{r}")
                nc.sync.dma_start(
                    rt[:], pvec_all[r * 512:(r + 1) * 512]
                    .rearrange("(o a) -> o a", o=1))
                pvec_rows.append(rt)

            # persistent scan state + zeroed padded buffers + A/Bc tiles
            s_r = spool.tile([128, 512], f32, tag="sr")
            s_i = spool.tile([128, 512], f32, tag="si")
            a_r = spool.tile([128, 512], f32, tag="a_r")
            a_i = spool.tile([128, 512], f32, tag="a_i")
            bc_r = spool.tile([128, 512], f32, tag="bc_r")
            bc_i = spool.tile([128, 512], f32, tag="bc_i")
            upad = spool.tile([64, NPAD], f16, tag="upad")
            cpad = spool.tile([128, NPAD], f16, tag="cpad")
            ds_all = spool.tile([1, L], f32, tag="ds_all")
            rmax = spool.tile([128, 1], f32, tag="rmax")
            nc.vector.memset(upad[:], 0.0)
            nc.vector.memset(cpad[:], 0.0)
            nc.vector.memset(rmax[:], 0.0)

            # ---------------- device prologue: A_bar / B_coeff ----------------
            def bcast(row):
                pb = pw_pool.tile([128, 512], f32, tag="pw")
                nc.tensor.matmul(pb[:], cst["ones1"][:],
                                 pvec_rows[row][:], start=True, stop=True)
                dst = spool.tile([128, 512], f32, tag=f"bc_row{row}")
                nc.scalar.copy(dst[:], pb[:])
                return dst

            lam_re_b = bcast(0)
            lam_im_b = bcast(1)
            step_b = bcast(2)
            kv0_b = bcast(3)
            kv1_b = bcast(4)
            kv2_b = bcast(5)

            t_d = spool.tile([128, 512], f32, tag="t_d")
            t_e = spool.tile([128, 512], f32, tag="t_e")
            tr = spool.tile([128, 512], f32, tag="t_tr")
            ti = spool.tile([128, 512], f32, tag="t_ti")
            # D = (kv2*CW + kv1)*CH + kv0*CW + 1
            nc.vector.tensor_mul(t_d[:], kv2_b[:], cst["cw_pk"][:])
            nc.vector.tensor_add(t_d[:], t_d[:], kv1_b[:])
            nc.vector.tensor_mul(t_d[:], t_d[:], cst["ch_pk"][:])
            nc.vector.tensor_mul(t_e[:], kv0_b[:], cst["cw_pk"][:])
            nc.vector.tensor_add(t_d[:], t_d[:], t_e[:])
            nc.vector.tensor_scalar_add(t_d[:], t_d[:], 1.0)
            # temp = lam * D (complex); m = temp*step
            nc.vector.tensor_mul(tr[:], lam_re_b[:], t_d[:])
            nc.vector.tensor_mul(ti[:], lam_im_b[:], t_d[:])
            mr = lam_re_b   # reuse row tiles as scratch
            mi = lam_im_b
            nc.vector.tensor_mul(mr[:], tr[:], step_b[:])
            nc.vector.tensor_mul(mi[:], ti[:], step_b[:])
            # A^(1/32) = exp(mr/32) * (1-2*sin(mi/64)^2, sin(mi/32))
            er32 = step_b   # scratch
            s32 = kv0_b
            s64 = kv1_b
            nc.scalar.activation(er32[:], mr[:], AF.Exp, scale=1.0 / 32.0)
            nc.scalar.activation(s32[:], mi[:], AF.Sin, scale=1.0 / 32.0)
            nc.scalar.activation(s64[:], mi[:], AF.Sin, scale=1.0 / 64.0)
            nc.vector.tensor_mul(t_e[:], s64[:], s64[:])
            nc.vector.tensor_scalar(t_e[:], t_e[:], -2.0, 1.0,
                                    op0=mybir.AluOpType.mult,
                                    op1=mybir.AluOpType.add)   # c32
            nc.vector.tensor_mul(a_r[:], er32[:], t_e[:])
            nc.vector.tensor_mul(a_i[:], er32[:], s32[:])
            # 5x complex squaring -> A_bar
            sq_r = kv2_b    # scratch
            sq_i = t_e
            for _ in range(5):
                nc.vector.tensor_mul(sq_r[:], a_r[:], a_r[:])
                nc.vector.tensor_mul(sq_i[:], a_i[:], a_i[:])
                nc.vector.tensor_mul(a_i[:], a_i[:], a_r[:])
                nc.vector.tensor_sub(a_r[:], sq_r[:], sq_i[:])
                nc.vector.tensor_add(a_i[:], a_i[:], a_i[:])
            # B_coeff = (A-1) * conj(temp) / |temp|^2
            d2 = mr         # scratch
            nc.vector.tensor_mul(d2[:], tr[:], tr[:])
            nc.vector.tensor_mul(t_d[:], ti[:], ti[:])
            nc.vector.tensor_add(d2[:], d2[:], t_d[:])
            inv = mi        # scratch
            nc.vector.reciprocal(inv[:], d2[:])
            arm1 = sq_r
            nc.vector.tensor_scalar_add(arm1[:], a_r[:], -1.0)
            nc.vector.tensor_mul(bc_r[:], arm1[:], tr[:])
            nc.vector.tensor_mul(t_d[:], a_i[:], ti[:])
            nc.vector.tensor_add(bc_r[:], bc_r[:], t_d[:])
            nc.vector.tensor_mul(bc_r[:], bc_r[:], inv[:])
            nc.vector.tensor_mul(bc_i[:], a_i[:], tr[:])
            nc.vector.tensor_mul(t_d[:], arm1[:], ti[:])
            nc.vector.tensor_sub(bc_i[:], bc_i[:], t_d[:])
            nc.vector.tensor_mul(bc_i[:], bc_i[:], inv[:])

            # ---------------- conv / transform helpers ----------------
            def load_and_pad(src_ap, dst_pad, nch):
                """DRAM int8 flat (1024*nch,) -> dst_pad (nch, 1156) channel-major.

                The int8 payload is u / s_core; the dequant scale is folded
                into the B/D conv weights host-side."""
                u0 = work.tile([128, 8, nch], i8, tag="u0")
                nc.sync.dma_start(
                    u0[:], src_ap.rearrange("(t q u) -> q t u", q=128, u=nch))
                u0h = work.tile([128, 8, nch], f16, tag="u0h")
                nc.scalar.copy(u0h[:], u0[:])
                for t in range(8):
                    pt = pt_pool.tile([nch, 128], f16, tag="pt")
                    nc.tensor.transpose(pt[:], u0h[:, t, :], cst["ident16"][:])
                    pv = dst_pad.rearrange("c (r w) -> c r w", w=PW)
                    nc.scalar.copy(pv[:, 4 * t + 1:4 * t + 5, 1:33], pt[:])
                u2a = work.tile([128, NPAD], f16, tag="u2a")
                u2b = work.tile([128, NPAD], f16, tag="u2b")
                nc.gpsimd.tensor_copy(u2a[0:64, :], dst_pad[:])
                nc.gpsimd.tensor_copy(u2a[64:128, 0:NPAD - 1], dst_pad[:, 1:])
                nc.gpsimd.tensor_copy(u2b[0:64, :], dst_pad[:])
                nc.gpsimd.tensor_copy(u2b[64:128, 0:NPAD - 34], dst_pad[:, 34:])
                return u2a, u2b

            def conv_paired_into(psum_out, wp_tile, ws_tile, u2a, u2b, pad_tile,
                                 start, stop):
                """5-group paired conv accumulate: psum_out (cout, 512) x2 chunks."""
                va = u2a.rearrange("c (r w) -> c r w", w=PW)
                vb = u2b.rearrange("c (r w) -> c r w", w=PW)
                vs = pad_tile.rearrange("c (r w) -> c r w", w=PW)
                for c2 in range(2):
                    for g in range(4):
                        kh, kw = PAIR_OFFS[g] // PW, PAIR_OFFS[g] % PW
                        pv = va if PAIR_DELTA[g] == 1 else vb
                        nc.tensor.matmul(
                            psum_out[:, bass.ts(c2, 512)], wp_tile[:, g, :],
                            pv[:, kh + 16 * c2:kh + 16 * c2 + 16, kw:kw + 32],
                            start=(start and g == 0), stop=False)
                    nc.tensor.matmul(
                        psum_out[:, bass.ts(c2, 512)], ws_tile[:],
                        vs[:, 2 + 16 * c2:2 + 16 * c2 + 16, 2:34],
                        start=False, stop=stop)

            def fwd_stage(bu_ps):
                """bu_ps PSUM (128, 1024) -> (bhr, bhi) PSUM (128, 512) each."""
                s1 = work.tile([128, 1024], f32, tag="s1")
                nc.scalar.copy(s1[:, 0:512], bu_ps[:, 0:512])
                nc.scalar.copy(s1[:, 512:1024], bu_ps[:, 512:1024])
                t1 = work.tile([128, 8, 128], f32, tag="t1")
                for t in range(8):
                    pt = pt_pool.tile([128, 128], f32, tag="pt")
                    nc.tensor.transpose(pt[:], s1[:, bass.ts(t, 128)], cst["ident"][:])
                    nc.scalar.copy(t1[:, t, :], pt[:])
                rr = t1[:, :, 0:64]
                ri = t1[:, :, 64:128]
                yr = pw_pool.tile([128, 512], f32, tag="pw")
                yi = pw_pool.tile([128, 512], f32, tag="pw")
                nc.tensor.matmul(yr[:], cst["qwf_r"][:], rr, start=True, stop=False)
                nc.tensor.matmul(yr[:], cst["qwf_ni"][:], ri, start=False, stop=True)
                nc.tensor.matmul(yi[:], cst["qwf_i"][:], rr, start=True, stop=False)
                nc.tensor.matmul(yi[:], cst["qwf_r"][:], ri, start=False, stop=True)
                yw = work.tile([128, 8, 128], f32, tag="yw")
                nc.scalar.copy(yw[:, :, 0:64], yr[:].rearrange("p (t f) -> p t f", t=8))
                nc.scalar.copy(yw[:, :, 64:128], yi[:].rearrange("p (t f) -> p t f", t=8))
                z = work.tile([128, 1024], f32, tag="z")
                zv = z.rearrange("p (c tb dh) -> p c tb dh", tb=8, dh=4)
                for t in range(8):
                    pt = pt_pool.tile([128, 128], f32, tag="pt")
                    nc.tensor.transpose(pt[:], yw[:, t, :], cst["ident"][:])
                    nc.scalar.copy(zv[:, :, t, :],
                                   pt.rearrange("p (dh c) -> p c dh", dh=4))
                t2 = work.tile([128, 8, 128], f32, tag="t2")
                for uu in range(8):
                    pt = pt_pool.tile([128, 128], f32, tag="pt")
                    nc.tensor.transpose(pt[:], z[:, bass.ts(uu, 128)], cst["ident"][:])
                    nc.scalar.copy(t2[:, uu, :], pt[:])
                xr = t2[:, :, 0:64]
                xi = t2[:, :, 64:128]
                bhr = pbh_pool.tile([128, 512], f32, tag="pbh")
                bhi = pbh_pool.tile([128, 512], f32, tag="pbh")
                nc.tensor.matmul(bhr[:], cst["qhf_r"][:], xr, start=True, stop=False)
                nc.tensor.matmul(bhr[:], cst["qhf_ni"][:], xi, start=False, stop=True)
                nc.tensor.matmul(bhi[:], cst["qhf_i"][:], xr, start=True, stop=False)
                nc.tensor.matmul(bhi[:], cst["qhf_r"][:], xi, start=False, stop=True)
                return bhr, bhi

            def full_fwd(src_ap):
                u2a, u2b = load_and_pad(src_ap, upad, 64)
                bu = pacc.tile([128, 1024], f32, tag="pacc")
                conv_paired_into(bu, cst["wbp"], cst["wbs"], u2a, u2b, upad,
                                 start=True, stop=True)
                return fwd_stage(bu), u2a, u2b

            # ---- prologue: x0 ----
            (bhr0, bhi0), _, _ = full_fwd(seg8("x0", NPIX * U))
            q1 = tmp_pool.tile([128, 512], f32, tag="q1")
            q2 = tmp_pool.tile([128, 512], f32, tag="q2")
            nc.vector.tensor_mul(q1[:], bc_r[:], bhr0[:])
            nc.vector.tensor_mul(q2[:], bc_i[:], bhi0[:])
            nc.vector.tensor_sub(s_r[:], q1[:], q2[:])
            nc.vector.tensor_mul(q1[:], bc_r[:], bhi0[:])
            nc.vector.tensor_mul(q2[:], bc_i[:], bhr0[:])
            nc.vector.tensor_add(s_i[:], q1[:], q2[:])

            # ---- main loop ----
            for l in range(L):
                off_l = BLOB_OFF["useq"] + l * NPIX * U
                (bhr, bhi), u2a_l, u2b_l = full_fwd(
                    wire(off_l, NPIX * U))
                # scan update (DVE)
                t_a = tmp_pool.tile([128, 512], f32, tag="q1")
                t_b = tmp_pool.tile([128, 512], f32, tag="q2")
                t_c = tmp_pool.tile([128, 512], f32, tag="q3")
                t_dd = tmp_pool.tile([128, 512], f32, tag="q4")
                nr = tmp_pool.tile([128, 512], f32, tag="nr")
                nc.vector.tensor_mul(t_a[:], a_r[:], s_r[:])
                nc.vector.tensor_mul(t_b[:], a_i[:], s_i[:])
                nc.vector.tensor_sub(t_a[:], t_a[:], t_b[:])
                nc.vector.tensor_mul(t_c[:], bc_r[:], bhr[:])
                nc.vector.tensor_mul(t_dd[:], bc_i[:], bhi[:])
                nc.vector.tensor_sub(t_c[:], t_c[:], t_dd[:])
                nc.vector.tensor_add(nr[:], t_a[:], t_c[:])
                nc.vector.tensor_mul(t_a[:], a_r[:], s_i[:])
                nc.vector.tensor_mul(t_b[:], a_i[:], s_r[:])
                nc.vector.tensor_add(t_a[:], t_a[:], t_b[:])
                nc.vector.tensor_mul(t_c[:], bc_r[:], bhi[:])
                nc.vector.tensor_mul(t_dd[:], bc_i[:], bhr[:])
                nc.vector.tensor_add(t_c[:], t_c[:], t_dd[:])
                nc.vector.tensor_add(s_i[:], t_a[:], t_c[:])
                nc.vector.tensor_copy(s_r[:], nr[:])

                # inverse transform
                x1r = pw_pool.tile([128, 512], f32, tag="pw")
                x1i = pw_pool.tile([128, 512], f32, tag="pw")
                nc.tensor.matmul(x1r[:], cst["qhi_r"][:], s_r[:], start=True, stop=False)
                nc.tensor.matmul(x1r[:], cst["qhi_ni"][:], s_i[:], start=False, stop=True)
                nc.tensor.matmul(x1i[:], cst["qhi_i"][:], s_r[:], start=True, stop=False)
                nc.tensor.matmul(x1i[:], cst["qhi_r"][:], s_i[:], start=False, stop=True)
                xs1 = work.tile([128, 8, 128], f32, tag="xs1")
                nc.scalar.copy(xs1[:, :, 0:64], x1r[:].rearrange("p (t f) -> p t f", t=8))
                nc.scalar.copy(xs1[:, :, 64:128], x1i[:].rearrange("p (t f) -> p t f", t=8))
                zi = work.tile([128, 1024], f32, tag="zi")
                ziv = zi.rearrange("p (h ub dc) -> p h ub dc", ub=8, dc=4)
                for uu in range(8):
                    pt = pt_pool.tile([128, 128], f32, tag="pt")
                    nc.tensor.transpose(pt[:], xs1[:, uu, :], cst["ident"][:])
                    nc.scalar.copy(ziv[:, :, uu, :],
                                   pt.rearrange("p (dc h) -> p h dc", dc=4))
                t2i = work.tile([128, 8, 128], f32, tag="t2i")
                for vv in range(8):
                    pt = pt_pool.tile([128, 128], f32, tag="pt")
                    nc.tensor.transpose(pt[:], zi[:, bass.ts(vv, 128)], cst["ident"][:])
                    nc.scalar.copy(t2i[:, vv, :], pt[:])
                wr = t2i[:, :, 0:64]
                wi = t2i[:, :, 64:128]
                xspr = pw_pool.tile([128, 512], f32, tag="pw")
                xspi = pw_pool.tile([128, 512], f32, tag="pw")
                nc.tensor.matmul(xspr[:], cst["qwi_r"][:], wr, start=True, stop=False)
                nc.tensor.matmul(xspr[:], cst["qwi_ni"][:], wi, start=False, stop=True)
                nc.tensor.matmul(xspi[:], cst["qwi_i"][:], wr, start=True, stop=False)
                nc.tensor.matmul(xspi[:], cst["qwi_r"][:], wi, start=False, stop=True)
                xsp = work.tile([128, 8, 128], f32, tag="xsp")
                nc.scalar.copy(xsp[:, :, 0:64], xspr[:].rearrange("p (t f) -> p t f", t=8))
                nc.scalar.copy(xsp[:, :, 64:128], xspi[:].rearrange("p (t f) -> p t f", t=8))
                for vv in range(8):
                    pt = pt_pool.tile([128, 128], f32, tag="pt")
                    nc.tensor.transpose(
                        pt[:], xsp[:, vv, :], cst["ident"][:])
                    cv = cpad.rearrange("c (r w) -> c r w", w=PW)
                    nc.scalar.copy(cv[:, 4 * vv + 1:4 * vv + 5, 1:33], pt[:])
                # C conv + D conv into one PSUM, then gelu
                yps = pacc.tile([64, 1024], f32, tag="pacc")
                cpv = cpad.rearrange("c (r w) -> c r w", w=PW)
                for c2 in range(2):
                    for tidx, (kh, kw) in enumerate(taps):
                        nc.tensor.matmul(
                            yps[:, bass.ts(c2, 512)], cst["wc"][:, tidx, :],
                            cpv[:, kh + 16 * c2:kh + 16 * c2 + 16, kw:kw + 32],
                            start=(tidx == 0), stop=False)
                conv_paired_into(yps, cst["wdp"], cst["wds"], u2a_l, u2b_l, upad,
                                 start=False, stop=True)
                yout = work.tile([64, 1024], f32, tag="yout")
                nc.scalar.activation(yout[:], yps[:], AF.Gelu_apprx_tanh)
                # per-l int8 quantization scale: qs = 127 / max|yout|
                nc.vector.tensor_reduce(rmax[0:64, :], yout[:],
                                        axis=mybir.AxisListType.X,
                                        op=mybir.AluOpType.max,
                                        apply_absolute_value=True)
                gall = tmp_pool.tile([128, 1], f32, tag="gall")
                nc.gpsimd.partition_all_reduce(
                    gall[:], rmax[:], channels=128,
                    reduce_op=bass_isa.ReduceOp.max)
                nc.vector.tensor_scalar_max(gall[:], gall[:], 1e-6)
                q1s = tmp_pool.tile([128, 1], f32, tag="q1s")
                nc.vector.tensor_scalar_mul(q1s[:], gall[:], 1.0 / 127.0)
                nc.scalar.copy(ds_all[:, l:l + 1], q1s[0:1, :])
                qsb = tmp_pool.tile([128, 1], f32, tag="qsb")
                nc.vector.reciprocal(qsb[:], q1s[:])
                osb = work.tile([128, 8, 64], i8, tag="osb")
                for t in range(8):
                    pt = pt_pool.tile([128, 64], f32, tag="pt")
                    nc.tensor.transpose(
                        pt[:], yout[:, bass.ts(t, 128)], cst["ident"][:64, :64])
                    nc.scalar.activation(osb[:, t, :], pt[:], AF.Copy,
                                         scale=qsb[:])
                ytgt = yq_d[l // (L // 4)][l % (L // 4)]
                nc.sync.dma_start(
                    ytgt.rearrange("(t q) u -> q t u", q=128), osb[:])
            nc.sync.dma_start(ysc_d[:], ds_all[:])
    nc.finalize()
    return nc


# ----------------------------------------------------------------------------
# Custom cached runner (bass_exec via PJRT, no retrace, no zero buffers)
# ----------------------------------------------------------------------------

_CACHE = {}


def _get_runner():
    if "fn" in _CACHE:
        return _CACHE["fn"]
    import jax
    import concourse.mybir as mybir
    from concourse import bass2jax
    from jax.sharding import Mesh, PartitionSpec
    from jax.experimental.shard_map import shard_map

    nc = build_bass()
    assert nc.dbg_addr is None
    bass2jax.install_neuronx_cc_hook()

    partition_name = nc.partition_id_tensor.name if nc.partition_id_tensor else None
    in_names, out_names, out_avals = [], [], []
    for alloc in nc.m.functions[0].allocations:
        if not isinstance(alloc, mybir.MemoryLocationSet):
            continue
        name = alloc.memorylocations[0].name
        if alloc.kind == "ExternalInput":
            if name != partition_name:
                in_names.append(name)
        elif alloc.kind == "ExternalOutput":
            out_names.append(name)
            out_avals.append(jax.core.ShapedArray(tuple(alloc.tensor_shape),
                                                  mybir.dt.np(alloc.dtype)))
    user_in_names = list(in_names)
    if partition_name is not None:
        in_names.append(partition_name)

    def _body(*args):
        operands = list(args)
        if partition_name is not None:
            operands.append(bass2jax.partition_id_tensor())
        outs = bass2jax._bass_exec_p.bind(
            *operands,
            out_avals=tuple(out_avals),
            in_names=tuple(in_names),
            out_names=tuple(out_names),
            lowering_input_output_aliases=(),
            sim_require_finite=True,
            sim_require_nnan=True,
            nc=nc,
        )
        return tuple(outs)

    devices = jax.devices()[:BSZ]
    mesh = Mesh(np.asarray(devices), ("core",))
    in_avals = []
    for alloc in nc.m.functions[0].allocations:
        if not isinstance(alloc, mybir.MemoryLocationSet):
            continue
        name = alloc.memorylocations[0].name
        if alloc.kind == "ExternalInput" and name in user_in_names:
            shp = list(alloc.tensor_shape)
            shp[0] *= BSZ
            in_avals.append(
                jax.ShapeDtypeStruct(tuple(shp), mybir.dt.np(alloc.dtype)))

    def _compile():
        return jax.jit(
            shard_map(_body, mesh=mesh,
                      in_specs=(PartitionSpec("core"),) * len(user_in_names),
                      out_specs=(PartitionSpec("core"),) * len(out_names),
                      check_rep=False),
            keep_unused=True,
        ).lower(*in_avals).compile()

    try:
        fn = bass2jax.fast_dispatch_compile(_compile)
    except Exception:
        fn = jax.jit(
            shard_map(_body, mesh=mesh,
                      in_specs=(PartitionSpec("core"),) * len(user_in_names),
                      out_specs=(PartitionSpec("core"),) * len(out_names),
                      check_rep=False),
            keep_unused=True,
        )
    from jax.sharding import NamedSharding
    _CACHE["devices"] = devices
    _CACHE["sharding"] = NamedSharding(mesh, PartitionSpec("core"))
    _CACHE["fn"] = (fn, user_in_names, out_names)
    return _CACHE["fn"]


def kernel(**inputs):
    import jax
    from jax.sharding import NamedSharding, PartitionSpec

    fn, user_in_names, out_names = _get_runner()
    inputs = {k: np.asarray(v) for k, v in inputs.items()}

    dyn = pack_dynamic(
        inputs["Lambda_re"], inputs["Lambda_im"], inputs["values"],
        inputs["log_step"], inputs["B_r"], inputs["B_i"], inputs["C_r"],
        inputs["C_i"], inputs["D_kernel"])
    w32 = {k: dyn[k].astype(np.float32).reshape(-1)
           for k in ("wbp", "wdp", "wbs", "wds")}
    wc16 = dyn["wc"].reshape(-1)
    pv32 = dyn["pvec"].reshape(-1)

    # useq/x0 ship as int8 (per-core scale folded into the B/D conv weight
    # copies); each core's wire blob is packed and device_put independently
    # so packing overlaps the serial axon uploads.
    useq = inputs["input_sequence"].reshape(L, BSZ, NPIX, U)
    x0 = inputs["x0"].reshape(BSZ, NPIX, U)
    devices = _CACHE["devices"]
    sharding = _CACHE["sharding"]

    # reuse per-core wire buffers and f32 scratch across calls: avoids ~20MB
    # of fresh page-faulted allocations inside the timed call
    if "blob_bufs" not in _CACHE:
        _CACHE["blob_bufs"] = [np.empty(NWIRE, dtype=np.int8)
                               for _ in range(BSZ)]
        _CACHE["scratch"] = [np.empty((L, NPIX, U), dtype=np.float32)
                             for _ in range(BSZ)]

    def _pack_core(b):
        ub, xb = useq[:, b], x0[b]
        s = np.float32(max(np.abs(ub).max(), np.abs(xb).max()) / 127.0)
        blob = _CACHE["blob_bufs"][b]
        tmp = _CACHE["scratch"][b]
        o, n = BLOB_OFF["useq"], L * NPIX * U
        np.multiply(ub, np.float32(1.0) / s, out=tmp)
        np.rint(tmp, out=tmp)
        np.copyto(blob[o:o + n].reshape(L, NPIX, U), tmp, casting="unsafe")
        o, n = BLOB_OFF["x0"], NPIX * U
        np.copyto(blob[o:o + n].reshape(NPIX, U),
                  np.rint(xb * (np.float32(1.0) / s)), casting="unsafe")
        for k, v in w32.items():
            o = BLOB_OFF[k]
            np.copyto(blob[o:o + 2 * v.size].view(np.float16), v * s,
                      casting="unsafe")
        o = BLOB_OFF["wc"]
        blob[o:o + 2 * wc16.size].view(np.float16)[:] = wc16
        o = BLOB_OFF["pvec"]
        blob[o:o + 4 * pv32.size].view(np.float32)[:] = pv32
        return (jax.device_put(blob[:NSPLIT], devices[b]),
                jax.device_put(blob[NSPLIT:], devices[b]))

    NSPLIT = (L // 2) * NPIX * U
    from concurrent.futures import ThreadPoolExecutor
    with ThreadPoolExecutor(max_workers=8) as ex:
        shards = list(ex.map(_pack_core, range(BSZ)))
    garr_a = jax.make_array_from_single_device_arrays(
        (BSZ * NSPLIT,), sharding, [s[0] for s in shards])
    garr_b = jax.make_array_from_single_device_arrays(
        (BSZ * (NWIRE - NSPLIT),), sharding, [s[1] for s in shards])
    args = {"bloba": garr_a, "blobb": garr_b}
    args = [args[name] for name in user_in_names]
    try:
        outs = fn(*args)
    except Exception:
        # transient device hiccups (e.g. NRT exec-unit recovery) — retry once
        import time as _time
        _time.sleep(2.0)
        outs = fn(*args)
    y_arrs = [outs[out_names.index(f"y{i}")] for i in range(4)]
    ysc_arr = outs[out_names.index("ysc")]

    # fetch + dequantize shards in parallel: concurrent D2H requests keep
    # the axon pipe busy, each shard is processed as soon as its device is
    # done, and the int8->f32 multiplies run in the threads
    try:
        for a in y_arrs:
            a.copy_to_host_async()
        ysc_arr.copy_to_host_async()
    except Exception:
        pass
    out = np.empty((L, BSZ, NPIX, U), dtype=np.float32)
    LH = L // 4

    sc_by_b = {(s.index[0].start or 0): s for s in ysc_arr.addressable_shards}

    def _fetch_sc(b):
        return b, np.asarray(sc_by_b[b].data)[0]            # (L,) f32

    def _fetch_dequant(job):
        yshard, l0 = job
        b = yshard.index[0].start // LH if yshard.index[0].start else 0
        sc = sc_host[b]
        data = np.asarray(yshard.data)                      # (LH, NPIX, U) int8
        np.multiply(data, sc[l0:l0 + LH].reshape(LH, 1, 1),
                    out=out[l0:l0 + LH, b], casting="unsafe")

    jobs = [(s, i * LH) for i, a in enumerate(y_arrs)
            for s in a.addressable_shards]
    with ThreadPoolExecutor(max_workers=32) as ex:
        sc_host = dict(ex.map(_fetch_sc, range(BSZ)))
        list(ex.map(_fetch_dequant, jobs))
    return out.reshape(L, BSZ, H, W, U)


# ----------------------------------------------------------------------------
# Numpy mock of the device pipeline (for layout validation): run with
# `python test.py mock`. Mirrors the device ops in f32.
# ----------------------------------------------------------------------------

def host_constants_mock(Lambda_re, Lambda_im, values, log_step, B_r, B_i,
                        C_r, C_i, D_kernel):
    scst = static_constants()
    dyn = pack_dynamic(Lambda_re, Lambda_im, values, log_step, B_r, B_i,
                       C_r, C_i, D_kernel)
    ar, ai, bcr, bci = abc_host(dyn["pvec"].astype(np.float64), scst)
    cst = dict(scst)
    cst.update({k: v.astype(np.float32) for k, v in dyn.items()})
    cst.update({"a_r": ar, "a_i": ai, "bc_r": bcr, "bc_i": bci})
    return cst


def _mock_core(useq, x0, cst):
    """useq (L, 1024, 64), x0 (1024, 64) -> y (L, 1024, 64). Mirrors device ops."""
    taps = [(kh, kw) for kh in range(3) for kw in range(3)]

    def pad_cm(img_cm):  # (ch, 1024) -> (ch, 34*34) zero border
        nch = img_cm.shape[0]
        p = np.zeros((nch, PW, PW), dtype=np.float32)
        p[:, 1:33, 1:33] = img_cm.reshape(nch, 32, 32)
        return p.reshape(nch, NPAD)

    def conv_cm(pad, wk):  # pad (cin, 1156), wk (cin, 9, cout) -> (cout, 1024)
        acc = np.zeros((wk.shape[2], NPIX), dtype=np.float32)
        for t, (kh, kw) in enumerate(taps):
            win = pad.reshape(-1, PW, PW)[:, kh:kh + 32, kw:kw + 32].reshape(-1, NPIX)
            acc += wk[:, t, :].astype(np.float32).T @ win
        return acc

    def win_of(buf, off):  # buf (nch, 1156) -> strided window (nch, 1024)
        v = np.zeros((buf.shape[0], 16 * 2, 32), dtype=np.float32)
        for c2 in range(2):
            for r in range(16):
                s = off + (16 * c2 + r) * PW
                v[:, 16 * c2 + r, :] = buf[:, s:s + 32]
        return v.reshape(buf.shape[0], NPIX)

    def conv_paired(up2d1, up2d34, upad_, wp, ws):
        acc = np.zeros((wp.shape[2], NPIX), dtype=np.float32)
        for g in range(4):
            buf = up2d1 if PAIR_DELTA[g] == 1 else up2d34
            acc += wp[:, g, :].astype(np.float32).T @ win_of(buf, PAIR_OFFS[g])
        acc += ws.astype(np.float32).T @ win_of(upad_, SOLO_OFF)
        return acc

    def fwd_transform(bu_cm):  # (128=[r|i]p, 1024 pix) -> bhr, bhi (128, 512)
        t1 = np.zeros((128, 8, 128), dtype=np.float32)
        for t in range(8):
            t1[:, t, :] = bu_cm[:, t * 128:(t + 1) * 128].T
        rr = t1[:, :, 0:64].reshape(128, 512)
        ri = t1[:, :, 64:128].reshape(128, 512)
        yr = cst["qwf_r"].T @ rr + cst["qwf_ni"].T @ ri
        yi = cst["qwf_i"].T @ rr + cst["qwf_r"].T @ ri
        y = np.zeros((128, 8, 2, 64), dtype=np.float32)
        y[:, :, 0, :] = yr.reshape(128, 8, 64)
        y[:, :, 1, :] = yi.reshape(128, 8, 64)
        z2 = np.zeros((128, 32, 8, 4), dtype=np.float32)   # (ch, c, hb, dh)
        for t in range(8):
            pt = y[:, t, :, :].reshape(128, 128).T         # (ch, (dh, c))
            z2[:, :, t, :] = pt.reshape(128, 4, 32).transpose(0, 2, 1)
        z2 = z2.reshape(128, 1024)
        t2 = np.zeros((128, 8, 128), dtype=np.float32)
        for uu in range(8):
            t2[:, uu, :] = z2[:, uu * 128:(uu + 1) * 128].T
        xr = t2[:, :, 0:64].reshape(128, 512)
        xi = t2[:, :, 64:128].reshape(128, 512)
        bhr = cst["qhf_r"].T @ xr + cst["qhf_ni"].T @ xi
        bhi = cst["qhf_i"].T @ xr + cst["qhf_r"].T @ xi
        return bhr, bhi

    def inv_transform(sr, si):  # scan state (128,512) -> xsp (128=[r|i]p, 1024 pix)
        x1r = cst["qhi_r"].T @ sr + cst["qhi_ni"].T @ si
        x1i = cst["qhi_i"].T @ sr + cst["qhi_r"].T @ si
        xs1 = np.zeros((128, 8, 2, 64), dtype=np.float32)
        xs1[:, :, 0, :] = x1r.reshape(128, 8, 64)
        xs1[:, :, 1, :] = x1i.reshape(128, 8, 64)
        z2i = np.zeros((128, 32, 8, 4), dtype=np.float32)  # (ch, h, ub, dc)
        for uu in range(8):
            pt = xs1[:, uu, :, :].reshape(128, 128).T      # (ch, (dc, h))
            z2i[:, :, uu, :] = pt.reshape(128, 4, 32).transpose(0, 2, 1)
        z2i = z2i.reshape(128, 1024)
        t2i = np.zeros((128, 8, 128), dtype=np.float32)
        for vv in range(8):
            t2i[:, vv, :] = z2i[:, vv * 128:(vv + 1) * 128].T
        wr = t2i[:, :, 0:64].reshape(128, 512)
        wi = t2i[:, :, 64:128].reshape(128, 512)
        xspr = cst["qwi_r"].T @ wr + cst["qwi_ni"].T @ wi
        xspi = cst["qwi_i"].T @ wr + cst["qwi_r"].T @ wi
        xsp = np.zeros((128, 8, 2, 64), dtype=np.float32)
        xsp[:, :, 0, :] = xspr.reshape(128, 8, 64)
        xsp[:, :, 1, :] = xspi.reshape(128, 8, 64)
        out = np.zeros((128, 1024), dtype=np.float32)      # (ch=[r|i]p, pix)
        for vv in range(8):
            out[:, vv * 128:(vv + 1) * 128] = xsp[:, vv, :].reshape(128, 128).T
        return out

    def fwd_from_img(img):  # (1024, 64) -> bhr, bhi
        up = pad_cm(img.T.astype(np.float32))
        up2d1 = np.zeros((128, NPAD), dtype=np.float32)
        up2d1[0:64] = up
        up2d1[64:128, 0:NPAD - 1] = up[:, 1:]
        up2d34 = np.zeros((128, NPAD), dtype=np.float32)
        up2d34[0:64] = up
        up2d34[64:128, 0:NPAD - 34] = up[:, 34:]
        bu = conv_paired(up2d1, up2d34, up, cst["wbp"], cst["wbs"])
        return fwd_transform(bu), (up, up2d1, up2d34)

    y_out = np.zeros((L, NPIX, U), dtype=np.float32)
    (bhr0, bhi0), _ = fwd_from_img(x0)
    sr = cst["bc_r"] * bhr0 - cst["bc_i"] * bhi0
    si = cst["bc_r"] * bhi0 + cst["bc_i"] * bhr0
    for l in range(L):
        (bhr, bhi), upad = fwd_from_img(useq[l])
        nsr = (cst["a_r"] * sr - cst["a_i"] * si) + (cst["bc_r"] * bhr - cst["bc_i"] * bhi)
        nsi = (cst["a_r"] * si + cst["a_i"] * sr) + (cst["bc_r"] * bhi + cst["bc_i"] * bhr)
        sr, si = nsr, nsi
        xsp = inv_transform(sr, si)                        # (128, 1024)
        cpad = pad_cm(xsp)                                 # (128, 1156)
        up_, u2d1_, u2d34_ = upad
        yacc = conv_cm(cpad, cst["wc"]) + conv_paired(u2d1_, u2d34_, up_, cst["wdp"], cst["wds"])
        g = 0.5 * yacc * (1.0 + np.tanh(0.7978845608028654 * (yacc + 0.044715 * yacc ** 3)))
        y_out[l] = g.T
    return y_out


def mock_kernel(**inputs):
    cst = host_constants_mock(
        inputs["Lambda_re"], inputs["Lambda_im"], inputs["values"], inputs["log_step"],
        inputs["B_r"], inputs["B_i"], inputs["C_r"], inputs["C_i"], inputs["D_kernel"])
    useq = inputs["input_sequence"].reshape(L, BSZ, NPIX, U)
    x0 = inputs["x0"].reshape(BSZ, NPIX, U)
    outs = [_mock_core(useq[:, b], x0[b], cst) for b in range(BSZ)]
    return np.stack(outs, axis=1).reshape(L, BSZ, H, W, U)


# revision 57
# speedup vs baseline: 1.0668x; 1.0266x over previous
"""Trainium2 Bass kernel for the BTST-SSM problem.

Math: 2D state-space model. Per l: u -> conv(B) -> DST-eigendomain ->
diagonal linear recurrence over l -> inverse DST -> conv(C) + conv(D) -> gelu.

Sharding: batch (8) across 8 cores, one sample per core. No collectives.

The warm call is axon-tunnel-bandwidth bound (~30-40 MB/s, half duplex), so
the design minimizes wire bytes and per-RPC overheads:
  - useq/x0 ship as int8 (per-core scale folded into that core's B/D conv
    weight copies; dequant on device is a free int8->fp16 copy)
  - conv weights ship as fp16; y returns as int8 with per-l scales computed
    on device (vector reduce_max + gpsimd partition_all_reduce)
  - all per-core inputs ride in ONE int8 wire tensor (fp16/f32 segments via
    AP.bitcast), packed per core in threads that overlap the serial uploads
  - A_bar/B_coeff are computed on device from tiny per-channel vectors:
    A^(1/32) via Exp/Sin (args stay in the accurate range) then 5 complex
    squarings; B = (A-1)*conj(temp)/|temp|^2 with vector.reciprocal
  - input-independent DST/basis constants are baked into the NEFF via
    inline_tensor (shipped once at executable load)
  - custom cached-jit runner built directly on _bass_exec_p: no per-call
    retrace, no zero donation buffers, fast-dispatch compile
  - y shards are fetched + dequantized concurrently (8 threads)

Per-core layouts (SBUF tiles are (128 partitions, free)):
  - channel-major image: (ch, h*w) padded to (ch, 34*34) for SAME conv
  - spatial tiles for transforms: partition = (g4, x32) block-diag groups
  - scan state: partition q = dc*32 + a  (c = 4u + dc), free col = u*64 + p
All transform matrices are packed host-side as kron(I4, blk) lhsT tensors;
complex arithmetic is done with +/- weight copies accumulated in PSUM.

Accuracy: ~1.43e-2 max-relative (dominated by int8 input quantization,
validated against the reference in the numpy mock; deterministic).
"""

import os
import sys
import numpy as np

sys.path.insert(0, "/opt/trn_rl_repo")

H = W = 32
P = 64
U = 64
L = 32
BSZ = 8
PW = 34          # padded width/height
NPIX = H * W     # 1024
NPAD = PW * PW   # 1156
PI = float(np.pi)


# ----------------------------------------------------------------------------
# Host constant computation (float64 -> float32)
# ----------------------------------------------------------------------------

def _dst_q(n):
    idx = np.arange(1, n + 1, dtype=np.float64)
    s = np.sin(np.pi * idx[:, None] * idx[None, :] / (n + 1)) / np.sqrt((n + 1) / 2.0)
    phase = np.exp(1j * (np.pi / 2.0) * idx)
    return phase[:, None] * s


def _softmax(x, axis):
    m = np.max(x, axis=axis, keepdims=True)
    e = np.exp(x - m)
    return e / np.sum(e, axis=axis, keepdims=True)


def static_constants():
    """Input-independent constants baked into the NEFF as inline tensors."""
    Qh = _dst_q(H)
    Qw = _dst_q(W)
    Qh_inv = np.conj(Qh).T
    Qw_inv = np.conj(Qw).T

    def kr(m):
        return np.kron(np.eye(4), m).astype(np.float32)

    ih = np.arange(1, H + 1, dtype=np.float64)
    iw = np.arange(1, W + 1, dtype=np.float64)
    ch = 2.0 * np.cos(np.pi * ih / (H + 1))
    cw = 2.0 * np.cos(np.pi * iw / (W + 1))
    # packed (q=(dc,a), col=(u,p)) grids of cw[c]=cw[4u+dc] and ch[a]
    cw_pk = np.zeros((128, 512), dtype=np.float64)
    ch_pk = np.zeros((128, 512), dtype=np.float64)
    for dc in range(4):
        for uu in range(8):
            cw_pk[dc * 32:(dc + 1) * 32, uu * 64:(uu + 1) * 64] = cw[4 * uu + dc]
    for a in range(32):
        ch_pk[a::32, :] = ch[a]

    return {
        "qwf_r": kr(np.real(Qw_inv).T), "qwf_i": kr(np.imag(Qw_inv).T),
        "qwf_ni": kr(-np.imag(Qw_inv).T),
        "qhf_r": kr(np.real(Qh_inv).T), "qhf_i": kr(np.imag(Qh_inv).T),
        "qhf_ni": kr(-np.imag(Qh_inv).T),
        "qhi_r": kr(np.real(Qh).T), "qhi_i": kr(np.imag(Qh).T),
        "qhi_ni": kr(-np.imag(Qh).T),
        "qwi_r": kr(np.real(Qw).T), "qwi_i": kr(np.imag(Qw).T),
        "qwi_ni": kr(-np.imag(Qw).T),
        "ident": np.eye(128, dtype=np.float32),
        "ident16": np.eye(128, dtype=np.float16),
        "ones1": np.ones((1, 128), dtype=np.float32),
        "cw_pk": cw_pk.astype(np.float32),
        "ch_pk": ch_pk.astype(np.float32),
    }


PAIRS = [((0, 0), (0, 1)), ((1, 0), (1, 1)), ((2, 0), (2, 1)), ((0, 2), (1, 2))]
PAIR_OFFS = [0, 34, 68, 2]      # window offset of t0 per pair group
PAIR_DELTA = [1, 1, 1, 34]      # o(t1) - o(t0); delta 1 -> up2d1, 34 -> up2d34
SOLO_OFF = 70                   # (2,2)

# single int8 wire blob per core: quantized useq+x0 (per-core scale folded
# into that core's B/D conv weights), fp16 conv weights, and f32 pvec — one
# large axon transfer instead of many. Offsets are in BYTES, 64-aligned.
_SEGS8 = [("useq", L * NPIX * U), ("x0", NPIX * U)]          # int8, 1B each
_SEGS16 = [("wbp", 128 * 4 * 128), ("wdp", 128 * 4 * 64), ("wbs", 64 * 128),
           ("wds", 64 * 64), ("wc", 128 * 9 * 64)]           # fp16, 2B each
BLOB_OFF = {}
_o = 0
for _n, _s in _SEGS8:
    BLOB_OFF[_n] = _o
    _o += _s
for _n, _s in _SEGS16:
    BLOB_OFF[_n] = _o
    _o += 2 * _s
BLOB_OFF["pvec"] = _o
_o += 6 * 512 * 4
NWIRE = _o


def pack_dynamic(Lambda_re, Lambda_im, values, log_step, B_r, B_i, C_r, C_i,
                 D_kernel):
    """Per-call small tensors: channel vectors + fp16-packed conv weights."""
    lam_re = np.minimum(Lambda_re.astype(np.float64), -1e-4)
    lam_im = Lambda_im.astype(np.float64)
    step = np.exp(log_step.astype(np.float64))
    v = _softmax(values.astype(np.float64), axis=-1) * 4.0
    xk, yk, zk, wk = v[:, 0], v[:, 1], v[:, 2], v[:, 3]
    kv = np.stack(((xk + yk - 2) / 4, (xk + zk - 2) / 4, (xk + wk - 2) / 8),
                  axis=-1)                                     # (P, 3)
    pvec = np.zeros((6, 512), dtype=np.float64)
    rows = [lam_re, lam_im, step, kv[:, 0], kv[:, 1], kv[:, 2]]
    for r, vec in enumerate(rows):
        pvec[r] = np.tile(vec, 8)
    pvec = pvec.astype(np.float32)

    wb = np.concatenate([B_r, B_i], axis=-1).transpose(2, 0, 1, 3) \
        .reshape(U, 9, 2 * P)
    wc = np.concatenate([2.0 * C_r, -2.0 * C_i], axis=2).transpose(2, 0, 1, 3) \
        .reshape(2 * P, 9, U).astype(np.float16)
    wd = D_kernel.transpose(2, 0, 1, 3).reshape(U, 9, U)
    wb9 = wb.reshape(U, 3, 3, 2 * P)
    wd9 = wd.reshape(U, 3, 3, U)

    def pack_pairs(w9, cout):
        out = np.zeros((128, 4, cout), dtype=np.float16)
        for g, (t0, t1) in enumerate(PAIRS):
            out[0:64, g, :] = w9[:, t0[0], t0[1], :]
            out[64:128, g, :] = w9[:, t1[0], t1[1], :]
        return out

    return {
        "pvec": pvec,
        "wbp": pack_pairs(wb9, 2 * P),
        "wdp": pack_pairs(wd9, U),
        "wbs": np.ascontiguousarray(wb9[:, 2, 2, :]).astype(np.float16),
        "wds": np.ascontiguousarray(wd9[:, 2, 2, :]).astype(np.float16),
        "wc": wc,
    }


# ----------------------------------------------------------------------------
# Host reference of the device A_bar/B_coeff computation (for validation)
# ----------------------------------------------------------------------------

def abc_host(pvec, cst):
    """Mirror of the device prologue, in numpy f32."""
    lam_re = pvec[0][None, :]      # broadcast rows (128, 512)
    lam_im = pvec[1][None, :]
    step = pvec[2][None, :]
    kv0, kv1, kv2 = pvec[3][None, :], pvec[4][None, :], pvec[5][None, :]
    CW, CH = cst["cw_pk"], cst["ch_pk"]
    D = (kv2 * CW + kv1) * CH + kv0 * CW + 1.0
    tr = lam_re * D
    ti = lam_im * D
    mr = tr * step
    mi = ti * step
    er32 = np.exp(mr / 32.0)
    s32 = np.sin(mi / 32.0)
    s64 = np.sin(mi / 64.0)
    c32 = 1.0 - 2.0 * s64 * s64
    ar, ai = er32 * c32, er32 * s32
    for _ in range(5):
        ar, ai = ar * ar - ai * ai, 2.0 * ar * ai
    d2 = tr * tr + ti * ti
    inv = 1.0 / d2
    arm1 = ar - 1.0
    bc_r = (arm1 * tr + ai * ti) * inv
    bc_i = (ai * tr - arm1 * ti) * inv
    return ar, ai, bc_r, bc_i


# ----------------------------------------------------------------------------
# Bass kernel
# ----------------------------------------------------------------------------

def build_bass():
    import concourse.bass as bass
    import concourse.bacc as bacc
    import concourse.mybir as mybir
    import concourse.tile as tile
    import concourse.bass_isa as bass_isa

    f32 = mybir.dt.float32
    f16 = mybir.dt.float16
    AF = mybir.ActivationFunctionType
    nc = bacc.Bacc(None)

    i8 = mybir.dt.int8
    # wire blob splits into two input tensors per core so each core exposes
    # two H2D buffers (more in-flight upload RPCs on the axon pipe)
    NSPLIT = (L // 2) * NPIX * U        # first half of useq
    bloba_d = nc.dram_tensor("bloba", [NSPLIT], i8, kind="ExternalInput")
    blobb_d = nc.dram_tensor("blobb", [NWIRE - NSPLIT], i8,
                             kind="ExternalInput")

    def wire(off, size):
        if off + size <= NSPLIT:
            return bloba_d[off:off + size]
        assert off >= NSPLIT
        return blobb_d[off - NSPLIT:off - NSPLIT + size]

    def seg8(name, size):
        return wire(BLOB_OFF[name], size)

    def seg16(name, size):
        return wire(BLOB_OFF[name], 2 * size).bitcast(f16)

    def seg32(name, size):
        return wire(BLOB_OFF[name], 4 * size).bitcast(f32)

    dyn_shapes = {
        "wbp": (128, 4, 128), "wdp": (128, 4, 64),
        "wbs": (64, 128), "wds": (64, 64),
        "wc": (128, 9, 64),
    }
    # y splits into two output tensors so each core exposes two D2H buffers
    # (more in-flight fetch RPCs on the axon pipe)
    ya_d = nc.dram_tensor("ya", [L // 2, NPIX, U], i8, kind="ExternalOutput")
    yb_d = nc.dram_tensor("yb", [L // 2, NPIX, U], i8, kind="ExternalOutput")
    ysc_d = nc.dram_tensor("ysc", [1, L], f32, kind="ExternalOutput")

    scst = static_constants()
    cst_dram = {k: nc.inline_tensor(v, name=k) for k, v in scst.items()}

    taps = [(kh, kw) for kh in range(3) for kw in range(3)]

    with tile.TileContext(nc) as tc:
        with (
            tc.tile_pool(name="cpool", bufs=1) as cpool,
            tc.tile_pool(name="state", bufs=1) as spool,
            tc.tile_pool(name="work", bufs=2) as work,
            tc.tile_pool(name="tmp", bufs=2) as tmp_pool,
            tc.tile_pool(name="pacc", bufs=1, space="PSUM") as pacc,
            tc.tile_pool(name="pt", bufs=2, space="PSUM") as pt_pool,
            tc.tile_pool(name="pw", bufs=2, space="PSUM") as pw_pool,
            tc.tile_pool(name="pbh", bufs=2, space="PSUM") as pbh_pool,
        ):
            cst = {}
            for k, v in scst.items():
                t = cpool.tile(list(v.shape), f16 if v.dtype == np.float16 else f32,
                               tag=k, name=k)
                nc.sync.dma_start(t[:], cst_dram[k][:])
                cst[k] = t
            for k, shp in dyn_shapes.items():
                t = cpool.tile(list(shp), f16, tag=k, name=k)
                nelem = int(np.prod(shp))
                src = seg16(k, nelem)
                if len(shp) == 2:
                    src = src.rearrange("(p a) -> p a", p=shp[0])
                else:
                    src = src.rearrange("(p a b) -> p a b", p=shp[0], a=shp[1])
                nc.sync.dma_start(t[:], src)
                cst[k] = t
            pvec_all = seg32("pvec", 6 * 512)
            pvec_rows = []
            for r in range(6):
                rt = cpool.tile([1, 512], f32, tag=f"pvec{r}", name=f"pvec_t{r}")
                nc.sync.dma_start(
                    rt[:], pvec_all[r * 512:(r + 1) * 512]
                    .rearrange("(o a) -> o a", o=1))
                pvec_rows.append(rt)

            # persistent scan state + zeroed padded buffers + A/Bc tiles
            s_r = spool.tile([128, 512], f32, tag="sr")
            s_i = spool.tile([128, 512], f32, tag="si")
            a_r = spool.tile([128, 512], f32, tag="a_r")
            a_i = spool.tile([128, 512], f32, tag="a_i")
            bc_r = spool.tile([128, 512], f32, tag="bc_r")
            bc_i = spool.tile([128, 512], f32, tag="bc_i")
            upad = spool.tile([64, NPAD], f16, tag="upad")
            cpad = spool.tile([128, NPAD], f16, tag="cpad")
            ds_all = spool.tile([1, L], f32, tag="ds_all")
            rmax = spool.tile([128, 1], f32, tag="rmax")
            nc.vector.memset(upad[:], 0.0)
            nc.vector.memset(cpad[:], 0.0)
            nc.vector.memset(rmax[:], 0.0)

            # ---------------- device prologue: A_bar / B_coeff ----------------
            def bcast(row):
                pb = pw_pool.tile([128, 512], f32, tag="pw")
                nc.tensor.matmul(pb[:], cst["ones1"][:],
                                 pvec_rows[row][:], start=True, stop=True)
                dst = spool.tile([128, 512], f32, tag=f"bc_row{row}")
                nc.scalar.copy(dst[:], pb[:])
                return dst

            lam_re_b = bcast(0)
            lam_im_b = bcast(1)
            step_b = bcast(2)
            kv0_b = bcast(3)
            kv1_b = bcast(4)
            kv2_b = bcast(5)

            t_d = spool.tile([128, 512], f32, tag="t_d")
            t_e = spool.tile([128, 512], f32, tag="t_e")
            tr = spool.tile([128, 512], f32, tag="t_tr")
            ti = spool.tile([128, 512], f32, tag="t_ti")
            # D = (kv2*CW + kv1)*CH + kv0*CW + 1
            nc.vector.tensor_mul(t_d[:], kv2_b[:], cst["cw_pk"][:])
            nc.vector.tensor_add(t_d[:], t_d[:], kv1_b[:])
            nc.vector.tensor_mul(t_d[:], t_d[:], cst["ch_pk"][:])
            nc.vector.tensor_mul(t_e[:], kv0_b[:], cst["cw_pk"][:])
            nc.vector.tensor_add(t_d[:], t_d[:], t_e[:])
            nc.vector.tensor_scalar_add(t_d[:], t_d[:], 1.0)
            # temp = lam * D (complex); m = temp*step
            nc.vector.tensor_mul(tr[:], lam_re_b[:], t_d[:])
            nc.vector.tensor_mul(ti[:], lam_im_b[:], t_d[:])
            mr = lam_re_b   # reuse row tiles as scratch
            mi = lam_im_b
            nc.vector.tensor_mul(mr[:], tr[:], step_b[:])
            nc.vector.tensor_mul(mi[:], ti[:], step_b[:])
            # A^(1/32) = exp(mr/32) * (1-2*sin(mi/64)^2, sin(mi/32))
            er32 = step_b   # scratch
            s32 = kv0_b
            s64 = kv1_b
            nc.scalar.activation(er32[:], mr[:], AF.Exp, scale=1.0 / 32.0)
            nc.scalar.activation(s32[:], mi[:], AF.Sin, scale=1.0 / 32.0)
            nc.scalar.activation(s64[:], mi[:], AF.Sin, scale=1.0 / 64.0)
            nc.vector.tensor_mul(t_e[:], s64[:], s64[:])
            nc.vector.tensor_scalar(t_e[:], t_e[:], -2.0, 1.0,
                                    op0=mybir.AluOpType.mult,
                                    op1=mybir.AluOpType.add)   # c32
            nc.vector.tensor_mul(a_r[:], er32[:], t_e[:])
            nc.vector.tensor_mul(a_i[:], er32[:], s32[:])
            # 5x complex squaring -> A_bar
            sq_r = kv2_b    # scratch
            sq_i = t_e
            for _ in range(5):
                nc.vector.tensor_mul(sq_r[:], a_r[:], a_r[:])
                nc.vector.tensor_mul(sq_i[:], a_i[:], a_i[:])
                nc.vector.tensor_mul(a_i[:], a_i[:], a_r[:])
                nc.vector.tensor_sub(a_r[:], sq_r[:], sq_i[:])
                nc.vector.tensor_add(a_i[:], a_i[:], a_i[:])
            # B_coeff = (A-1) * conj(temp) / |temp|^2
            d2 = mr         # scratch
            nc.vector.tensor_mul(d2[:], tr[:], tr[:])
            nc.vector.tensor_mul(t_d[:], ti[:], ti[:])
            nc.vector.tensor_add(d2[:], d2[:], t_d[:])
            inv = mi        # scratch
            nc.vector.reciprocal(inv[:], d2[:])
            arm1 = sq_r
            nc.vector.tensor_scalar_add(arm1[:], a_r[:], -1.0)
            nc.vector.tensor_mul(bc_r[:], arm1[:], tr[:])
            nc.vector.tensor_mul(t_d[:], a_i[:], ti[:])
            nc.vector.tensor_add(bc_r[:], bc_r[:], t_d[:])
            nc.vector.tensor_mul(bc_r[:], bc_r[:], inv[:])
            nc.vector.tensor_mul(bc_i[:], a_i[:], tr[:])
            nc.vector.tensor_mul(t_d[:], arm1[:], ti[:])
            nc.vector.tensor_sub(bc_i[:], bc_i[:], t_d[:])
            nc.vector.tensor_mul(bc_i[:], bc_i[:], inv[:])

            # ---------------- conv / transform helpers ----------------
            def load_and_pad(src_ap, dst_pad, nch):
                """DRAM int8 flat (1024*nch,) -> dst_pad (nch, 1156) channel-major.

                The int8 payload is u / s_core; the dequant scale is folded
                into the B/D conv weights host-side."""
                u0 = work.tile([128, 8, nch], i8, tag="u0")
                nc.sync.dma_start(
                    u0[:], src_ap.rearrange("(t q u) -> q t u", q=128, u=nch))
                u0h = work.tile([128, 8, nch], f16, tag="u0h")
                nc.scalar.copy(u0h[:], u0[:])
                for t in range(8):
                    pt = pt_pool.tile([nch, 128], f16, tag="pt")
                    nc.tensor.transpose(pt[:], u0h[:, t, :], cst["ident16"][:])
                    pv = dst_pad.rearrange("c (r w) -> c r w", w=PW)
                    nc.scalar.copy(pv[:, 4 * t + 1:4 * t + 5, 1:33], pt[:])
                u2a = work.tile([128, NPAD], f16, tag="u2a")
                u2b = work.tile([128, NPAD], f16, tag="u2b")
                nc.gpsimd.tensor_copy(u2a[0:64, :], dst_pad[:])
                nc.gpsimd.tensor_copy(u2a[64:128, 0:NPAD - 1], dst_pad[:, 1:])
                nc.gpsimd.tensor_copy(u2b[0:64, :], dst_pad[:])
                nc.gpsimd.tensor_copy(u2b[64:128, 0:NPAD - 34], dst_pad[:, 34:])
                return u2a, u2b

            def conv_paired_into(psum_out, wp_tile, ws_tile, u2a, u2b, pad_tile,
                                 start, stop):
                """5-group paired conv accumulate: psum_out (cout, 512) x2 chunks."""
                va = u2a.rearrange("c (r w) -> c r w", w=PW)
                vb = u2b.rearrange("c (r w) -> c r w", w=PW)
                vs = pad_tile.rearrange("c (r w) -> c r w", w=PW)
                for c2 in range(2):
                    for g in range(4):
                        kh, kw = PAIR_OFFS[g] // PW, PAIR_OFFS[g] % PW
                        pv = va if PAIR_DELTA[g] == 1 else vb
                        nc.tensor.matmul(
                            psum_out[:, bass.ts(c2, 512)], wp_tile[:, g, :],
                            pv[:, kh + 16 * c2:kh + 16 * c2 + 16, kw:kw + 32],
                            start=(start and g == 0), stop=False)
                    nc.tensor.matmul(
                        psum_out[:, bass.ts(c2, 512)], ws_tile[:],
                        vs[:, 2 + 16 * c2:2 + 16 * c2 + 16, 2:34],
                        start=False, stop=stop)

            def fwd_stage(bu_ps):
                """bu_ps PSUM (128, 1024) -> (bhr, bhi) PSUM (128, 512) each."""
                s1 = work.tile([128, 1024], f32, tag="s1")
                nc.scalar.copy(s1[:, 0:512], bu_ps[:, 0:512])
                nc.scalar.copy(s1[:, 512:1024], bu_ps[:, 512:1024])
                t1 = work.tile([128, 8, 128], f32, tag="t1")
                for t in range(8):
                    pt = pt_pool.tile([128, 128], f32, tag="pt")
                    nc.tensor.transpose(pt[:], s1[:, bass.ts(t, 128)], cst["ident"][:])
                    nc.scalar.copy(t1[:, t, :], pt[:])
                rr = t1[:, :, 0:64]
                ri = t1[:, :, 64:128]
                yr = pw_pool.tile([128, 512], f32, tag="pw")
                yi = pw_pool.tile([128, 512], f32, tag="pw")
                nc.tensor.matmul(yr[:], cst["qwf_r"][:], rr, start=True, stop=False)
                nc.tensor.matmul(yr[:], cst["qwf_ni"][:], ri, start=False, stop=True)
                nc.tensor.matmul(yi[:], cst["qwf_i"][:], rr, start=True, stop=False)
                nc.tensor.matmul(yi[:], cst["qwf_r"][:], ri, start=False, stop=True)
                yw = work.tile([128, 8, 128], f32, tag="yw")
                nc.scalar.copy(yw[:, :, 0:64], yr[:].rearrange("p (t f) -> p t f", t=8))
                nc.scalar.copy(yw[:, :, 64:128], yi[:].rearrange("p (t f) -> p t f", t=8))
                z = work.tile([128, 1024], f32, tag="z")
                zv = z.rearrange("p (c tb dh) -> p c tb dh", tb=8, dh=4)
                for t in range(8):
                    pt = pt_pool.tile([128, 128], f32, tag="pt")
                    nc.tensor.transpose(pt[:], yw[:, t, :], cst["ident"][:])
                    nc.scalar.copy(zv[:, :, t, :],
                                   pt.rearrange("p (dh c) -> p c dh", dh=4))
                t2 = work.tile([128, 8, 128], f32, tag="t2")
                for uu in range(8):
                    pt = pt_pool.tile([128, 128], f32, tag="pt")
                    nc.tensor.transpose(pt[:], z[:, bass.ts(uu, 128)], cst["ident"][:])
                    nc.scalar.copy(t2[:, uu, :], pt[:])
                xr = t2[:, :, 0:64]
                xi = t2[:, :, 64:128]
                bhr = pbh_pool.tile([128, 512], f32, tag="pbh")
                bhi = pbh_pool.tile([128, 512], f32, tag="pbh")
                nc.tensor.matmul(bhr[:], cst["qhf_r"][:], xr, start=True, stop=False)
                nc.tensor.matmul(bhr[:], cst["qhf_ni"][:], xi, start=False, stop=True)
                nc.tensor.matmul(bhi[:], cst["qhf_i"][:], xr, start=True, stop=False)
                nc.tensor.matmul(bhi[:], cst["qhf_r"][:], xi, start=False, stop=True)
                return bhr, bhi

            def full_fwd(src_ap):
                u2a, u2b = load_and_pad(src_ap, upad, 64)
                bu = pacc.tile([128, 1024], f32, tag="pacc")
                conv_paired_into(bu, cst["wbp"], cst["wbs"], u2a, u2b, upad,
                                 start=True, stop=True)
                return fwd_stage(bu), u2a, u2b

            # ---- prologue: x0 ----
            (bhr0, bhi0), _, _ = full_fwd(seg8("x0", NPIX * U))
            q1 = tmp_pool.tile([128, 512], f32, tag="q1")
            q2 = tmp_pool.tile([128, 512], f32, tag="q2")
            nc.vector.tensor_mul(q1[:], bc_r[:], bhr0[:])
            nc.vector.tensor_mul(q2[:], bc_i[:], bhi0[:])
            nc.vector.tensor_sub(s_r[:], q1[:], q2[:])
            nc.vector.tensor_mul(q1[:], bc_r[:], bhi0[:])
            nc.vector.tensor_mul(q2[:], bc_i[:], bhr0[:])
            nc.vector.tensor_add(s_i[:], q1[:], q2[:])

            # ---- main loop ----
            for l in range(L):
                off_l = BLOB_OFF["useq"] + l * NPIX * U
                (bhr, bhi), u2a_l, u2b_l = full_fwd(
                    wire(off_l, NPIX * U))
                # scan update (DVE)
                t_a = tmp_pool.tile([128, 512], f32, tag="q1")
                t_b = tmp_pool.tile([128, 512], f32, tag="q2")
                t_c = tmp_pool.tile([128, 512], f32, tag="q3")
                t_dd = tmp_pool.tile([128, 512], f32, tag="q4")
                nr = tmp_pool.tile([128, 512], f32, tag="nr")
                nc.vector.tensor_mul(t_a[:], a_r[:], s_r[:])
                nc.vector.tensor_mul(t_b[:], a_i[:], s_i[:])
                nc.vector.tensor_sub(t_a[:], t_a[:], t_b[:])
                nc.vector.tensor_mul(t_c[:], bc_r[:], bhr[:])
                nc.vector.tensor_mul(t_dd[:], bc_i[:], bhi[:])
                nc.vector.tensor_sub(t_c[:], t_c[:], t_dd[:])
                nc.vector.tensor_add(nr[:], t_a[:], t_c[:])
                nc.vector.tensor_mul(t_a[:], a_r[:], s_i[:])
                nc.vector.tensor_mul(t_b[:], a_i[:], s_r[:])
                nc.vector.tensor_add(t_a[:], t_a[:], t_b[:])
                nc.vector.tensor_mul(t_c[:], bc_r[:], bhi[:])
                nc.vector.tensor_mul(t_dd[:], bc_i[:], bhr[:])
                nc.vector.tensor_add(t_c[:], t_c[:], t_dd[:])
                nc.vector.tensor_add(s_i[:], t_a[:], t_c[:])
                nc.vector.tensor_copy(s_r[:], nr[:])

                # inverse transform
                x1r = pw_pool.tile([128, 512], f32, tag="pw")
                x1i = pw_pool.tile([128, 512], f32, tag="pw")
                nc.tensor.matmul(x1r[:], cst["qhi_r"][:], s_r[:], start=True, stop=False)
                nc.tensor.matmul(x1r[:], cst["qhi_ni"][:], s_i[:], start=False, stop=True)
                nc.tensor.matmul(x1i[:], cst["qhi_i"][:], s_r[:], start=True, stop=False)
                nc.tensor.matmul(x1i[:], cst["qhi_r"][:], s_i[:], start=False, stop=True)
                xs1 = work.tile([128, 8, 128], f32, tag="xs1")
                nc.scalar.copy(xs1[:, :, 0:64], x1r[:].rearrange("p (t f) -> p t f", t=8))
                nc.scalar.copy(xs1[:, :, 64:128], x1i[:].rearrange("p (t f) -> p t f", t=8))
                zi = work.tile([128, 1024], f32, tag="zi")
                ziv = zi.rearrange("p (h ub dc) -> p h ub dc", ub=8, dc=4)
                for uu in range(8):
                    pt = pt_pool.tile([128, 128], f32, tag="pt")
                    nc.tensor.transpose(pt[:], xs1[:, uu, :], cst["ident"][:])
                    nc.scalar.copy(ziv[:, :, uu, :],
                                   pt.rearrange("p (dc h) -> p h dc", dc=4))
                t2i = work.tile([128, 8, 128], f32, tag="t2i")
                for vv in range(8):
                    pt = pt_pool.tile([128, 128], f32, tag="pt")
                    nc.tensor.transpose(pt[:], zi[:, bass.ts(vv, 128)], cst["ident"][:])
                    nc.scalar.copy(t2i[:, vv, :], pt[:])
                wr = t2i[:, :, 0:64]
                wi = t2i[:, :, 64:128]
                xspr = pw_pool.tile([128, 512], f32, tag="pw")
                xspi = pw_pool.tile([128, 512], f32, tag="pw")
                nc.tensor.matmul(xspr[:], cst["qwi_r"][:], wr, start=True, stop=False)
                nc.tensor.matmul(xspr[:], cst["qwi_ni"][:], wi, start=False, stop=True)
                nc.tensor.matmul(xspi[:], cst["qwi_i"][:], wr, start=True, stop=False)
                nc.tensor.matmul(xspi[:], cst["qwi_r"][:], wi, start=False, stop=True)
                xsp = work.tile([128, 8, 128], f32, tag="xsp")
                nc.scalar.copy(xsp[:, :, 0:64], xspr[:].rearrange("p (t f) -> p t f", t=8))
                nc.scalar.copy(xsp[:, :, 64:128], xspi[:].rearrange("p (t f) -> p t f", t=8))
                for vv in range(8):
                    pt = pt_pool.tile([128, 128], f32, tag="pt")
                    nc.tensor.transpose(
                        pt[:], xsp[:, vv, :], cst["ident"][:])
                    cv = cpad.rearrange("c (r w) -> c r w", w=PW)
                    nc.scalar.copy(cv[:, 4 * vv + 1:4 * vv + 5, 1:33], pt[:])
                # C conv + D conv into one PSUM, then gelu
                yps = pacc.tile([64, 1024], f32, tag="pacc")
                cpv = cpad.rearrange("c (r w) -> c r w", w=PW)
                for c2 in range(2):
                    for tidx, (kh, kw) in enumerate(taps):
                        nc.tensor.matmul(
                            yps[:, bass.ts(c2, 512)], cst["wc"][:, tidx, :],
                            cpv[:, kh + 16 * c2:kh + 16 * c2 + 16, kw:kw + 32],
                            start=(tidx == 0), stop=False)
                conv_paired_into(yps, cst["wdp"], cst["wds"], u2a_l, u2b_l, upad,
                                 start=False, stop=True)
                yout = work.tile([64, 1024], f32, tag="yout")
                nc.scalar.activation(yout[:], yps[:], AF.Gelu_apprx_tanh)
                # per-l int8 quantization scale: qs = 127 / max|yout|
                nc.vector.tensor_reduce(rmax[0:64, :], yout[:],
                                        axis=mybir.AxisListType.X,
                                        op=mybir.AluOpType.max,
                                        apply_absolute_value=True)
                gall = tmp_pool.tile([128, 1], f32, tag="gall")
                nc.gpsimd.partition_all_reduce(
                    gall[:], rmax[:], channels=128,
                    reduce_op=bass_isa.ReduceOp.max)
                nc.vector.tensor_scalar_max(gall[:], gall[:], 1e-6)
                q1s = tmp_pool.tile([128, 1], f32, tag="q1s")
                nc.vector.tensor_scalar_mul(q1s[:], gall[:], 1.0 / 127.0)
                nc.scalar.copy(ds_all[:, l:l + 1], q1s[0:1, :])
                qsb = tmp_pool.tile([128, 1], f32, tag="qsb")
                nc.vector.reciprocal(qsb[:], q1s[:])
                osb = work.tile([128, 8, 64], i8, tag="osb")
                for t in range(8):
                    pt = pt_pool.tile([128, 64], f32, tag="pt")
                    nc.tensor.transpose(
                        pt[:], yout[:, bass.ts(t, 128)], cst["ident"][:64, :64])
                    nc.scalar.activation(osb[:, t, :], pt[:], AF.Copy,
                                         scale=qsb[:])
                ytgt = ya_d[l] if l < L // 2 else yb_d[l - L // 2]
                nc.sync.dma_start(
                    ytgt.rearrange("(t q) u -> q t u", q=128), osb[:])
            nc.sync.dma_start(ysc_d[:], ds_all[:])
    nc.finalize()
    return nc


# ----------------------------------------------------------------------------
# Custom cached runner (bass_exec via PJRT, no retrace, no zero buffers)
# ----------------------------------------------------------------------------

_CACHE = {}


def _get_runner():
    if "fn" in _CACHE:
        return _CACHE["fn"]
    import jax
    import concourse.mybir as mybir
    from concourse import bass2jax
    from jax.sharding import Mesh, PartitionSpec
    from jax.experimental.shard_map import shard_map

    nc = build_bass()
    assert nc.dbg_addr is None
    bass2jax.install_neuronx_cc_hook()

    partition_name = nc.partition_id_tensor.name if nc.partition_id_tensor else None
    in_names, out_names, out_avals = [], [], []
    for alloc in nc.m.functions[0].allocations:
        if not isinstance(alloc, mybir.MemoryLocationSet):
            continue
        name = alloc.memorylocations[0].name
        if alloc.kind == "ExternalInput":
            if name != partition_name:
                in_names.append(name)
        elif alloc.kind == "ExternalOutput":
            out_names.append(name)
            out_avals.append(jax.core.ShapedArray(tuple(alloc.tensor_shape),
                                                  mybir.dt.np(alloc.dtype)))
    user_in_names = list(in_names)
    if partition_name is not None:
        in_names.append(partition_name)

    def _body(*args):
        operands = list(args)
        if partition_name is not None:
            operands.append(bass2jax.partition_id_tensor())
        outs = bass2jax._bass_exec_p.bind(
            *operands,
            out_avals=tuple(out_avals),
            in_names=tuple(in_names),
            out_names=tuple(out_names),
            lowering_input_output_aliases=(),
            sim_require_finite=True,
            sim_require_nnan=True,
            nc=nc,
        )
        return tuple(outs)

    devices = jax.devices()[:BSZ]
    mesh = Mesh(np.asarray(devices), ("core",))
    in_avals = []
    for alloc in nc.m.functions[0].allocations:
        if not isinstance(alloc, mybir.MemoryLocationSet):
            continue
        name = alloc.memorylocations[0].name
        if alloc.kind == "ExternalInput" and name in user_in_names:
            shp = list(alloc.tensor_shape)
            shp[0] *= BSZ
            in_avals.append(
                jax.ShapeDtypeStruct(tuple(shp), mybir.dt.np(alloc.dtype)))

    def _compile():
        return jax.jit(
            shard_map(_body, mesh=mesh,
                      in_specs=(PartitionSpec("core"),) * len(user_in_names),
                      out_specs=(PartitionSpec("core"),) * len(out_names),
                      check_rep=False),
            keep_unused=True,
        ).lower(*in_avals).compile()

    try:
        fn = bass2jax.fast_dispatch_compile(_compile)
    except Exception:
        fn = jax.jit(
            shard_map(_body, mesh=mesh,
                      in_specs=(PartitionSpec("core"),) * len(user_in_names),
                      out_specs=(PartitionSpec("core"),) * len(out_names),
                      check_rep=False),
            keep_unused=True,
        )
    from jax.sharding import NamedSharding
    _CACHE["devices"] = devices
    _CACHE["sharding"] = NamedSharding(mesh, PartitionSpec("core"))
    _CACHE["fn"] = (fn, user_in_names, out_names)
    return _CACHE["fn"]


def kernel(**inputs):
    import jax
    from jax.sharding import NamedSharding, PartitionSpec

    fn, user_in_names, out_names = _get_runner()
    inputs = {k: np.asarray(v) for k, v in inputs.items()}

    dyn = pack_dynamic(
        inputs["Lambda_re"], inputs["Lambda_im"], inputs["values"],
        inputs["log_step"], inputs["B_r"], inputs["B_i"], inputs["C_r"],
        inputs["C_i"], inputs["D_kernel"])
    w32 = {k: dyn[k].astype(np.float32).reshape(-1)
           for k in ("wbp", "wdp", "wbs", "wds")}
    wc16 = dyn["wc"].reshape(-1)
    pv32 = dyn["pvec"].reshape(-1)

    # useq/x0 ship as int8 (per-core scale folded into the B/D conv weight
    # copies); each core's wire blob is packed and device_put independently
    # so packing overlaps the serial axon uploads.
    useq = inputs["input_sequence"].reshape(L, BSZ, NPIX, U)
    x0 = inputs["x0"].reshape(BSZ, NPIX, U)
    devices = _CACHE["devices"]
    sharding = _CACHE["sharding"]

    # reuse per-core wire buffers and f32 scratch across calls: avoids ~20MB
    # of fresh page-faulted allocations inside the timed call
    if "blob_bufs" not in _CACHE:
        _CACHE["blob_bufs"] = [np.empty(NWIRE, dtype=np.int8)
                               for _ in range(BSZ)]
        _CACHE["scratch"] = [np.empty((L, NPIX, U), dtype=np.float32)
                             for _ in range(BSZ)]

    def _pack_core(b):
        ub, xb = useq[:, b], x0[b]
        s = np.float32(max(np.abs(ub).max(), np.abs(xb).max()) / 127.0)
        blob = _CACHE["blob_bufs"][b]
        tmp = _CACHE["scratch"][b]
        o, n = BLOB_OFF["useq"], L * NPIX * U
        np.multiply(ub, np.float32(1.0) / s, out=tmp)
        np.rint(tmp, out=tmp)
        np.copyto(blob[o:o + n].reshape(L, NPIX, U), tmp, casting="unsafe")
        o, n = BLOB_OFF["x0"], NPIX * U
        np.copyto(blob[o:o + n].reshape(NPIX, U),
                  np.rint(xb * (np.float32(1.0) / s)), casting="unsafe")
        for k, v in w32.items():
            o = BLOB_OFF[k]
            np.copyto(blob[o:o + 2 * v.size].view(np.float16), v * s,
                      casting="unsafe")
        o = BLOB_OFF["wc"]
        blob[o:o + 2 * wc16.size].view(np.float16)[:] = wc16
        o = BLOB_OFF["pvec"]
        blob[o:o + 4 * pv32.size].view(np.float32)[:] = pv32
        return (jax.device_put(blob[:NSPLIT], devices[b]),
                jax.device_put(blob[NSPLIT:], devices[b]))

    NSPLIT = (L // 2) * NPIX * U
    from concurrent.futures import ThreadPoolExecutor
    with ThreadPoolExecutor(max_workers=8) as ex:
        shards = list(ex.map(_pack_core, range(BSZ)))
    garr_a = jax.make_array_from_single_device_arrays(
        (BSZ * NSPLIT,), sharding, [s[0] for s in shards])
    garr_b = jax.make_array_from_single_device_arrays(
        (BSZ * (NWIRE - NSPLIT),), sharding, [s[1] for s in shards])
    args = {"bloba": garr_a, "blobb": garr_b}
    args = [args[name] for name in user_in_names]
    try:
        outs = fn(*args)
    except Exception:
        # transient device hiccups (e.g. NRT exec-unit recovery) — retry once
        import time as _time
        _time.sleep(2.0)
        outs = fn(*args)
    ya_arr = outs[out_names.index("ya")]
    yb_arr = outs[out_names.index("yb")]
    ysc_arr = outs[out_names.index("ysc")]

    # fetch + dequantize shards in parallel: concurrent D2H requests keep
    # the axon pipe busy, each shard is processed as soon as its device is
    # done, and the int8->f32 multiplies run in the threads
    try:
        ya_arr.copy_to_host_async()
        yb_arr.copy_to_host_async()
        ysc_arr.copy_to_host_async()
    except Exception:
        pass
    out = np.empty((L, BSZ, NPIX, U), dtype=np.float32)
    LH = L // 2

    sc_by_b = {(s.index[0].start or 0): s for s in ysc_arr.addressable_shards}

    def _fetch_sc(b):
        return b, np.asarray(sc_by_b[b].data)[0]            # (L,) f32

    def _fetch_dequant(job):
        yshard, l0 = job
        b = yshard.index[0].start // LH if yshard.index[0].start else 0
        sc = sc_host[b]
        data = np.asarray(yshard.data)                      # (LH, NPIX, U) int8
        np.multiply(data, sc[l0:l0 + LH].reshape(LH, 1, 1),
                    out=out[l0:l0 + LH, b], casting="unsafe")

    jobs = [(s, 0) for s in ya_arr.addressable_shards] + \
           [(s, LH) for s in yb_arr.addressable_shards]
    with ThreadPoolExecutor(max_workers=16) as ex:
        sc_host = dict(ex.map(_fetch_sc, range(BSZ)))
        list(ex.map(_fetch_dequant, jobs))
    return out.reshape(L, BSZ, H, W, U)


# ----------------------------------------------------------------------------
# Numpy mock of the device pipeline (for layout validation): run with
# `python test.py mock`. Mirrors the device ops in f32.
# ----------------------------------------------------------------------------

def host_constants_mock(Lambda_re, Lambda_im, values, log_step, B_r, B_i,
                        C_r, C_i, D_kernel):
    scst = static_constants()
    dyn = pack_dynamic(Lambda_re, Lambda_im, values, log_step, B_r, B_i,
                       C_r, C_i, D_kernel)
    ar, ai, bcr, bci = abc_host(dyn["pvec"].astype(np.float64), scst)
    cst = dict(scst)
    cst.update({k: v.astype(np.float32) for k, v in dyn.items()})
    cst.update({"a_r": ar, "a_i": ai, "bc_r": bcr, "bc_i": bci})
    return cst


def _mock_core(useq, x0, cst):
    """useq (L, 1024, 64), x0 (1024, 64) -> y (L, 1024, 64). Mirrors device ops."""
    taps = [(kh, kw) for kh in range(3) for kw in range(3)]

    def pad_cm(img_cm):  # (ch, 1024) -> (ch, 34*34) zero border
        nch = img_cm.shape[0]
        p = np.zeros((nch, PW, PW), dtype=np.float32)
        p[:, 1:33, 1:33] = img_cm.reshape(nch, 32, 32)
        return p.reshape(nch, NPAD)

    def conv_cm(pad, wk):  # pad (cin, 1156), wk (cin, 9, cout) -> (cout, 1024)
        acc = np.zeros((wk.shape[2], NPIX), dtype=np.float32)
        for t, (kh, kw) in enumerate(taps):
            win = pad.reshape(-1, PW, PW)[:, kh:kh + 32, kw:kw + 32].reshape(-1, NPIX)
            acc += wk[:, t, :].astype(np.float32).T @ win
        return acc

    def win_of(buf, off):  # buf (nch, 1156) -> strided window (nch, 1024)
        v = np.zeros((buf.shape[0], 16 * 2, 32), dtype=np.float32)
        for c2 in range(2):
            for r in range(16):
                s = off + (16 * c2 + r) * PW
                v[:, 16 * c2 + r, :] = buf[:, s:s + 32]
        return v.reshape(buf.shape[0], NPIX)

    def conv_paired(up2d1, up2d34, upad_, wp, ws):
        acc = np.zeros((wp.shape[2], NPIX), dtype=np.float32)
        for g in range(4):
            buf = up2d1 if PAIR_DELTA[g] == 1 else up2d34
            acc += wp[:, g, :].astype(np.float32).T @ win_of(buf, PAIR_OFFS[g])
        acc += ws.astype(np.float32).T @ win_of(upad_, SOLO_OFF)
        return acc

    def fwd_transform(bu_cm):  # (128=[r|i]p, 1024 pix) -> bhr, bhi (128, 512)
        t1 = np.zeros((128, 8, 128), dtype=np.float32)
        for t in range(8):
            t1[:, t, :] = bu_cm[:, t * 128:(t + 1) * 128].T
        rr = t1[:, :, 0:64].reshape(128, 512)
        ri = t1[:, :, 64:128].reshape(128, 512)
        yr = cst["qwf_r"].T @ rr + cst["qwf_ni"].T @ ri
        yi = cst["qwf_i"].T @ rr + cst["qwf_r"].T @ ri
        y = np.zeros((128, 8, 2, 64), dtype=np.float32)
        y[:, :, 0, :] = yr.reshape(128, 8, 64)
        y[:, :, 1, :] = yi.reshape(128, 8, 64)
        z2 = np.zeros((128, 32, 8, 4), dtype=np.float32)   # (ch, c, hb, dh)
        for t in range(8):
            pt = y[:, t, :, :].reshape(128, 128).T         # (ch, (dh, c))
            z2[:, :, t, :] = pt.reshape(128, 4, 32).transpose(0, 2, 1)
        z2 = z2.reshape(128, 1024)
        t2 = np.zeros((128, 8, 128), dtype=np.float32)
        for uu in range(8):
            t2[:, uu, :] = z2[:, uu * 128:(uu + 1) * 128].T
        xr = t2[:, :, 0:64].reshape(128, 512)
        xi = t2[:, :, 64:128].reshape(128, 512)
        bhr = cst["qhf_r"].T @ xr + cst["qhf_ni"].T @ xi
        bhi = cst["qhf_i"].T @ xr + cst["qhf_r"].T @ xi
        return bhr, bhi

    def inv_transform(sr, si):  # scan state (128,512) -> xsp (128=[r|i]p, 1024 pix)
        x1r = cst["qhi_r"].T @ sr + cst["qhi_ni"].T @ si
        x1i = cst["qhi_i"].T @ sr + cst["qhi_r"].T @ si
        xs1 = np.zeros((128, 8, 2, 64), dtype=np.float32)
        xs1[:, :, 0, :] = x1r.reshape(128, 8, 64)
        xs1[:, :, 1, :] = x1i.reshape(128, 8, 64)
        z2i = np.zeros((128, 32, 8, 4), dtype=np.float32)  # (ch, h, ub, dc)
        for uu in range(8):
            pt = xs1[:, uu, :, :].reshape(128, 128).T      # (ch, (dc, h))
            z2i[:, :, uu, :] = pt.reshape(128, 4, 32).transpose(0, 2, 1)
        z2i = z2i.reshape(128, 1024)
        t2i = np.zeros((128, 8, 128), dtype=np.float32)
        for vv in range(8):
            t2i[:, vv, :] = z2i[:, vv * 128:(vv + 1) * 128].T
        wr = t2i[:, :, 0:64].reshape(128, 512)
        wi = t2i[:, :, 64:128].reshape(128, 512)
        xspr = cst["qwi_r"].T @ wr + cst["qwi_ni"].T @ wi
        xspi = cst["qwi_i"].T @ wr + cst["qwi_r"].T @ wi
        xsp = np.zeros((128, 8, 2, 64), dtype=np.float32)
        xsp[:, :, 0, :] = xspr.reshape(128, 8, 64)
        xsp[:, :, 1, :] = xspi.reshape(128, 8, 64)
        out = np.zeros((128, 1024), dtype=np.float32)      # (ch=[r|i]p, pix)
        for vv in range(8):
            out[:, vv * 128:(vv + 1) * 128] = xsp[:, vv, :].reshape(128, 128).T
        return out

    def fwd_from_img(img):  # (1024, 64) -> bhr, bhi
        up = pad_cm(img.T.astype(np.float32))
        up2d1 = np.zeros((128, NPAD), dtype=np.float32)
        up2d1[0:64] = up
        up2d1[64:128, 0:NPAD - 1] = up[:, 1:]
        up2d34 = np.zeros((128, NPAD), dtype=np.float32)
        up2d34[0:64] = up
        up2d34[64:128, 0:NPAD - 34] = up[:, 34:]
        bu = conv_paired(up2d1, up2d34, up, cst["wbp"], cst["wbs"])
        return fwd_transform(bu), (up, up2d1, up2d34)

    y_out = np.zeros((L, NPIX, U), dtype=np.float32)
    (bhr0, bhi0), _ = fwd_from_img(x0)
    sr = cst["bc_r"] * bhr0 - cst["bc_i"] * bhi0
    si = cst["bc_r"] * bhi0 + cst["bc_i"] * bhr0
    for l in range(L):
        (bhr, bhi), upad = fwd_from_img(useq[l])
        nsr = (cst["a_r"] * sr - cst["a_i"] * si) + (cst["bc_r"] * bhr - cst["bc_i"] * bhi)
        nsi = (cst["a_r"] * si + cst["a_i"] * sr) + (cst["bc_r"] * bhi + cst["bc_i"] * bhr)
        sr, si = nsr, nsi
        xsp = inv_transform(sr, si)                        # (128, 1024)
        cpad = pad_cm(xsp)                                 # (128, 1156)
        up_, u2d1_, u2d34_ = upad
        yacc = conv_cm(cpad, cst["wc"]) + conv_paired(u2d1_, u2d34_, up_, cst["wdp"], cst["wds"])
        g = 0.5 * yacc * (1.0 + np.tanh(0.7978845608028654 * (yacc + 0.044715 * yacc ** 3)))
        y_out[l] = g.T
    return y_out


def mock_kernel(**inputs):
    cst = host_constants_mock(
        inputs["Lambda_re"], inputs["Lambda_im"], inputs["values"], inputs["log_step"],
        inputs["B_r"], inputs["B_i"], inputs["C_r"], inputs["C_i"], inputs["D_kernel"])
    useq = inputs["input_sequence"].reshape(L, BSZ, NPIX, U)
    x0 = inputs["x0"].reshape(BSZ, NPIX, U)
    outs = [_mock_core(useq[:, b], x0[b], cst) for b in range(BSZ)]
    return np.stack(outs, axis=1).reshape(L, BSZ, H, W, U)
